# revision 12
# baseline (speedup 1.0000x reference)
"""Grouped-query attention (GQA) Trainium2 Bass kernel, v2.

Problem: B=2, S=2048, DIM=2048, HQ=32, HKV=8, HEAD_DIM=64, causal mask.
Sharding: 8 cores = 2 (batch) x 4 (kv-head groups). Core c handles batch
c//4 and kv-block c%4 (2 kv heads, 8 q heads). Wq/Wk/Wv sharded
column-wise, Wo row-wise; each core writes a partial [S, DIM] output;
host sums the 4 partials per batch and adds bo.

v2 dataflow (all matmuls bf16 with fp32 PSUM accum):
  - q/k/v are transposed AND cast to bf16 on the HOST -> qT/kT/vT
    [DIM, S] in HBM. No on-chip input transposes or casts; DMA traffic
    halves vs f32 naturals.
  - Weights pre-cast to bf16 on host (columns of Wq / rows of Wo
    permuted so local q-head h sits in tile h%4 at partition (h//4)*64,
    matching its kv head's partition base in kxT).
  - Projections: kxT/vxT first (phase A), then per 512-row i-block:
    qxT, attention, output projection. Biases added on DVE
    (tensor_scalar) during PSUM->SBUF eviction.
  - Scores: the two kv heads of a q-head pair run as row-tiled
    concurrent matmuls (K=64 each, PE row halves 0-63 / 64-127) into
    one 2-bank PSUM tile; ONE Exp activation covers both (3-D AP skips
    the causally-masked tail). Triangular mask applied multiplicatively
    post-exp on diagonal j-blocks only; j-blocks above the diagonal are
    skipped entirely.
  - AV: stationary is [v_head (64 cols) | ones (64 cols)], so PSUM rows
    64:127 accumulate the softmax denominator replicated 64-wide.
    Normalization = full-width DVE reciprocal + 2 multiplies (no DMA
    broadcast, no single-partition ops).
  - Output projection: fp32 partial written straight from a [128, 2048]
    SBUF staging tile, 1 MiB per DMA.
"""

import numpy as np
import ml_dtypes

import concourse.bass as bass
import concourse.mybir as mybir
from concourse import bacc
from concourse.tile import TileContext
from concourse.bass_utils import run_bass_kernel_spmd

# This kernel uses Exp (softmax) and Ln (denominator reciprocal via
# exp(-ln(D))) on the ACT engine. The table-load placement pass maps each
# function to the first table set containing it, which picks
# `exp_and_others` for Exp and `natural_log` for Ln and thrashes
# ACT_TABLE_LOADs (~1.3us + drain each) on every normalization. Both live
# in `natural_log_exp_and_others`; narrow the claimed contents of the
# other sets (names and dict order - hence set ids - are unchanged) so
# the pass settles on the shared set once.
_orig_get_act_tables = bacc.get_activation_tables


def _pinned_act_tables(arch):
    tabs = _orig_get_act_tables(arch)
    exp = mybir.ActivationFunctionType.Exp
    ln = mybir.ActivationFunctionType.Ln
    shared = "natural_log_exp_and_others"
    if shared in tabs and exp in tabs[shared] and ln in tabs[shared]:
        for name, funcs in tabs.items():
            if name != shared:
                tabs[name] = funcs - {exp, ln}
    return tabs


bacc.get_activation_tables = _pinned_act_tables

F32 = mybir.dt.float32
BF16 = mybir.dt.bfloat16
AF = mybir.ActivationFunctionType
ALU = mybir.AluOpType

B, S, DIM = 2, 2048, 2048
HQ, HKV, HD = 32, 8, 64
GROUP = HQ // HKV              # 4
NCORES = 8
KVSH = 4                       # kv-blocks (shards) per batch
CQ = (HQ // KVSH) * HD         # 512 q-proj cols per core (8 heads)
CK = (HKV // KVSH) * HD        # 128 kv-proj cols per core (2 heads)
NDC = DIM // 128               # 16 contraction chunks
NSS = S // 512                 # 4 sequence chunks of 512
NJB = S // 128                 # 16 j-blocks of 128


def _ap3(sl, mid_stride, mid_n, last_n):
    """3-D AP over a 2-D tile slice: [partitions, mid_n x mid_stride,
    last_n] (element strides)."""
    return bass.AP(tensor=sl.tensor, offset=sl.offset,
                   ap=[list(sl.ap[0]), [mid_stride, mid_n], [1, last_n]])


def build_nc(mode="causal"):
    nc = bacc.Bacc("TRN2", target_bir_lowering=False)

    qT = nc.dram_tensor("qT", [DIM, S], BF16, kind="ExternalInput")
    kT = nc.dram_tensor("kT", [DIM, S], BF16, kind="ExternalInput")
    vT = nc.dram_tensor("vT", [DIM, S], BF16, kind="ExternalInput")
    wq = nc.dram_tensor("wq", [DIM, CQ], BF16, kind="ExternalInput")
    wk = nc.dram_tensor("wk", [DIM, CK], BF16, kind="ExternalInput")
    wv = nc.dram_tensor("wv", [DIM, CK], BF16, kind="ExternalInput")
    wo = nc.dram_tensor("wo", [CQ, DIM], BF16, kind="ExternalInput")
    bq = nc.dram_tensor("bq", [CQ], F32, kind="ExternalInput")
    bk = nc.dram_tensor("bk", [CK], F32, kind="ExternalInput")
    bv = nc.dram_tensor("bv", [CK], F32, kind="ExternalInput")
    tri2 = nc.dram_tensor("tri2", [128, 256], BF16, kind="ExternalInput")
    ident = nc.dram_tensor("ident", [128, 128], BF16, kind="ExternalInput")
    mbias = None
    if mode == "dense":
        mbias = nc.dram_tensor("mbias", [S, S], F32, kind="ExternalInput")
    out = nc.dram_tensor("out", [S, DIM], F32, kind="ExternalOutput")

    causal = mode == "causal"

    with TileContext(nc) as tc:
        with (
            tc.tile_pool(name="consts", bufs=1) as consts,
            tc.tile_pool(name="w", bufs=1) as wpool,
            tc.tile_pool(name="qt", bufs=1) as qtp,
            tc.tile_pool(name="stg", bufs=32) as stg,
            tc.tile_pool(name="acts", bufs=1) as acts,
            tc.tile_pool(name="vsb", bufs=2) as vxsb,
            tc.tile_pool(name="exp", bufs=3) as expp,
            tc.tile_pool(name="nm", bufs=2) as nmp,
            tc.tile_pool(name="ob", bufs=2) as obp,
            tc.tile_pool(name="mb", bufs=2) as mbp,
            tc.tile_pool(name="ps_sp", bufs=2, space="PSUM") as ps_sp,
            tc.tile_pool(name="ps_at", bufs=1, space="PSUM") as ps_at,
            tc.tile_pool(name="ps_pj", bufs=2, space="PSUM") as ps_pj,
        ):
            # ---- constants ----
            tri2_t = consts.tile([128, 256], BF16, tag="tri2")
            nc.sync.dma_start(out=tri2_t[:, :], in_=tri2[:, :])
            id_t = consts.tile([128, 128], BF16, tag="id")
            nc.sync.dma_start(out=id_t[:, :], in_=ident[:, :])
            bq_t = consts.tile([128, 4], F32, tag="bq")
            nc.sync.dma_start(
                out=bq_t[:, :],
                in_=bass.AP(tensor=bq[0:1].tensor, offset=0,
                            ap=[[1, 128], [128, 4]]))
            bk_t = consts.tile([128, 1], F32, tag="bk")
            nc.sync.dma_start(
                out=bk_t[:, :],
                in_=bass.AP(tensor=bk[0:1].tensor, offset=0,
                            ap=[[1, 128], [128, 1]]))
            bv_t = consts.tile([128, 1], F32, tag="bv")
            nc.sync.dma_start(
                out=bv_t[:, :],
                in_=bass.AP(tensor=bv[0:1].tensor, offset=0,
                            ap=[[1, 128], [128, 1]]))

            # ---- weights (already bf16) ----
            wq_t, wk_t, wv_t, wo_t = [], [], [], []
            for dc in range(NDC):
                t = wpool.tile([128, CK], BF16, tag=f"wk{dc}")
                nc.sync.dma_start(out=t[:, :],
                                  in_=wk[dc * 128:(dc + 1) * 128, :])
                wk_t.append(t)
                t = wpool.tile([128, CK], BF16, tag=f"wv{dc}")
                nc.sync.dma_start(out=t[:, :],
                                  in_=wv[dc * 128:(dc + 1) * 128, :])
                wv_t.append(t)
            for dc in range(NDC):
                t = wpool.tile([128, CQ], BF16, tag=f"wq{dc}")
                nc.sync.dma_start(out=t[:, :],
                                  in_=wq[dc * 128:(dc + 1) * 128, :])
                wq_t.append(t)
            # ---- qT loads: [128, 1024] per (dc, half); the second half
            # is emitted at its phase-B use point (slot grants follow
            # emission order, so an up-front emit would deadlock).
            # qT rides the HWDGE (sync) queue so it lands while the
            # SWDGE queue streams kT/vT for phase A. ----
            qT_t = {}

            def load_qT(sh):
                for dc in range(NDC):
                    t = qtp.tile([128, 1024], BF16, tag=f"qT{dc}",
                                 name=f"qT{dc}_{sh}")
                    nc.sync.dma_start(
                        out=t[:, :],
                        in_=qT[dc * 128:(dc + 1) * 128,
                               sh * 1024:(sh + 1) * 1024])
                    qT_t[(dc, sh)] = t

            load_qT(0)
            for cc in range(4):
                t = wpool.tile([128, DIM], BF16, tag=f"wo{cc}")
                nc.sync.dma_start(out=t[:, :],
                                  in_=wo[cc * 128:(cc + 1) * 128, :])
                wo_t.append(t)

            # ---- persistent activations ----
            qxT = {}   # (cc, ss) -> [128, 512]; rows 0:64 head cc (kv0),
            #            rows 64:128 head cc+4 (kv1)
            kxT = {}   # ss -> [128, 512]
            attnT = {}  # (pair, ss) -> [128, 512]
            for ss in range(NSS):
                kxT[ss] = acts.tile([128, 512], BF16, tag=f"kx{ss}", name=f"kx{ss}")
                for cc in range(4):
                    qxT[(cc, ss)] = acts.tile([128, 512], BF16,
                                              tag=f"qx{cc}_{ss}", name=f"qx{cc}_{ss}")
                    attnT[(cc, ss)] = acts.tile([128, 512], BF16,
                                                tag=f"at{cc}_{ss}", name=f"at{cc}_{ss}")
            # [v_head | 64 ones cols] per kv head, per j-block
            vx1r = []
            for jb in range(NJB):
                t = acts.tile([128, 256], BF16, tag=f"vp{jb}", name=f"vp{jb}")
                nc.vector.memset(t[:, 64:128], 1.0)
                nc.vector.memset(t[:, 192:256], 1.0)
                vx1r.append(t)

            # ---- phase B helpers ----
            def emit_g1q_cc(ss, cc):
                sh, lo = ss // 2, (ss % 2) * 512
                ps = ps_pj.tile([128, 512], F32, tag="pj")
                for dc in range(NDC):
                    nc.tensor.matmul(
                        ps[:, :], wq_t[dc][:, cc * 128:(cc + 1) * 128],
                        qT_t[(dc, sh)][:, lo:lo + 512],
                        start=(dc == 0), stop=(dc == NDC - 1))
                nc.vector.tensor_scalar_add(qxT[(cc, ss)][:, :],
                                            ps[:, :], bq_t[:, cc:cc + 1])

            def emit_g1q(ss):
                for cc in range(4):
                    emit_g1q_cc(ss, cc)

            def emit_attn_pair(ss, pair):
                s0 = ss * 512
                njb = 4 * (ss + 1) if causal else NJB
                qx = qxT[(pair, ss)]
                at = ps_at.tile([128, 1024], F32, tag="at")
                for jb in range(njb):
                    jss, jr = jb // 4, jb % 4
                    off = max(0, jb * 128 - s0) if causal else 0
                    N = 512 - off
                    sp = ps_sp.tile([128, 1024], F32, tag="sp")
                    nc.tensor.matmul(
                        sp[:, 0:N],
                        kxT[jss][0:64, jr * 128:(jr + 1) * 128],
                        qx[0:64, off:512], start=True, stop=True)
                    nc.tensor.matmul(
                        sp[:, 512:512 + N],
                        kxT[jss][64:128, jr * 128:(jr + 1) * 128],
                        qx[64:128, off:512], start=True, stop=True)
                    if mode == "dense":
                        mb = mbp.tile([128, 512], F32, tag="mb")
                        nc.sync.dma_start(
                            out=mb[:, 0:N],
                            in_=mbias[jb * 128:(jb + 1) * 128,
                                      s0 + off:s0 + 512])
                        nc.vector.tensor_tensor(
                            sp[:, 0:N], sp[:, 0:N], mb[:, 0:N], ALU.add)
                        nc.vector.tensor_tensor(
                            sp[:, 512:512 + N], sp[:, 512:512 + N],
                            mb[:, 0:N], ALU.add)
                    ex = expp.tile([128, 1024], BF16, tag="ex")
                    nc.scalar.activation(
                        _ap3(ex[:, 0:1024], 512, 2, N),
                        _ap3(sp[:, 0:1024], 512, 2, N),
                        AF.Exp, scale=0.125)
                    if causal and jss == ss:
                        nc.vector.tensor_tensor(
                            _ap3(ex[:, 0:1024], 512, 2, 128),
                            _ap3(ex[:, 0:1024], 512, 2, 128),
                            _ap3(tri2_t[:, 0:256], 128, 2, 128),
                            ALU.mult)
                    nc.tensor.matmul(
                        at[:, off:512], vx1r[jb][:, 0:128],
                        ex[:, 0:N],
                        start=(jb == 0), stop=(jb == njb - 1))
                    nc.tensor.matmul(
                        at[:, 512 + off:1024], vx1r[jb][:, 128:256],
                        ex[:, 512:512 + N],
                        start=(jb == 0), stop=(jb == njb - 1))
                # normalize: rows 64:128 hold the denominator replicated
                # 64-wide; 1/D = exp(-ln(D)) on ACT (Exp and Ln share one
                # table set; each call ~6x cheaper than DVE InstReciprocal)
                lnD = nmp.tile([64, 1024], F32, tag="nm")
                nc.scalar.activation(lnD[:, :], at[64:128, 0:1024], AF.Ln)
                nm = nmp.tile([64, 1024], F32, tag="nm")
                nc.scalar.activation(nm[:, :], lnD[:, :], AF.Exp,
                                     scale=-1.0)
                aT = attnT[(pair, ss)]
                nc.vector.tensor_tensor(
                    aT[0:64, :], at[0:64, 0:512], nm[0:64, 0:512],
                    ALU.mult)
                nc.vector.tensor_tensor(
                    aT[64:128, :], at[0:64, 512:1024],
                    nm[0:64, 512:1024], ALU.mult)

            def emit_g4(ss, ic):
                s0 = ss * 512
                i0 = ic * 128
                for hf in range(2):
                    ob = obp.tile([128, 1024], F32, tag="ob", name="ob")
                    for e2 in range(2):
                        ec = hf * 2 + e2
                        g4 = ps_pj.tile([128, 512], F32, tag="pj")
                        for cc2 in range(4):
                            nc.tensor.matmul(
                                g4[:, :],
                                attnT[(cc2, ss)][:, i0:i0 + 128],
                                wo_t[cc2][:, ec * 512:(ec + 1) * 512],
                                start=(cc2 == 0), stop=(cc2 == 3))
                        nc.vector.tensor_copy(
                            ob[:, e2 * 512:(e2 + 1) * 512], g4[:, :])
                    nc.sync.dma_start(
                        out=out[s0 + i0:s0 + i0 + 128,
                                hf * 1024:(hf + 1) * 1024],
                        in_=ob[:, :])

            # ---- phase A: k/v projections + v transpose ----
            for sh in range(2):
                ktl, vtl = {}, {}
                for dc in range(NDC):
                    t = stg.tile([128, 1024], BF16, tag="kv", name="kvstg")
                    nc.gpsimd.dma_start(
                        out=t[:, :],
                        in_=kT[dc * 128:(dc + 1) * 128,
                               sh * 1024:(sh + 1) * 1024])
                    ktl[dc] = t
                for dc in range(NDC):
                    t = stg.tile([128, 1024], BF16, tag="kv", name="kvstg")
                    nc.gpsimd.dma_start(
                        out=t[:, :],
                        in_=vT[dc * 128:(dc + 1) * 128,
                               sh * 1024:(sh + 1) * 1024])
                    vtl[dc] = t
                for ss in (2 * sh, 2 * sh + 1):
                    lo = (ss % 2) * 512
                    ps = ps_pj.tile([128, 512], F32, tag="pj")
                    for dc in range(NDC):
                        nc.tensor.matmul(ps[:, :], wk_t[dc][:, :],
                                         ktl[dc][:, lo:lo + 512],
                                         start=(dc == 0),
                                         stop=(dc == NDC - 1))
                    nc.vector.tensor_scalar_add(kxT[ss][:, :], ps[:, :],
                                                bk_t[:, 0:1])
                    ps = ps_pj.tile([128, 512], F32, tag="pj")
                    for dc in range(NDC):
                        nc.tensor.matmul(ps[:, :], wv_t[dc][:, :],
                                         vtl[dc][:, lo:lo + 512],
                                         start=(dc == 0),
                                         stop=(dc == NDC - 1))
                    vsb = vxsb.tile([128, 512], BF16, tag="vsb")
                    nc.vector.tensor_scalar_add(vsb[:, :], ps[:, :],
                                                bv_t[:, 0:1])
                    vtp = ps_pj.tile([128, 512], BF16, tag="pj")
                    for jr in range(4):
                        nc.tensor.transpose(vtp[:, jr * 128:(jr + 1) * 128],
                                            vsb[:, jr * 128:(jr + 1) * 128],
                                            id_t[:, :])
                    for jr in range(4):
                        jb = ss * 4 + jr
                        nc.vector.tensor_copy(
                            vx1r[jb][:, 0:64],
                            vtp[:, jr * 128:jr * 128 + 64])
                        nc.vector.tensor_copy(
                            vx1r[jb][:, 128:192],
                            vtp[:, jr * 128 + 64:jr * 128 + 128])
                    if sh == 0:
                        # qxT projections for i-blocks 0/1 fill the PE
                        # while SWDGE streams the sh=1 k/v tiles
                        emit_g1q(ss)

            # ---- phase B: GEMM4(ss-1) blocks and GEMM1q(ss+1) chains
            # emitted BETWEEN attention pairs of ss: tile-slot grants
            # follow emission order, so this is what lets ready
            # projection matmuls fill the PE idle slots where AV waits
            # on Exp ----
            load_qT(1)
            for ss in range(NSS):
                for pair in range(4):
                    emit_attn_pair(ss, pair)
                    if ss >= 1:
                        emit_g4(ss - 1, pair)
                    if 1 <= ss <= 2:
                        emit_g1q_cc(ss + 1, pair)
            for ic in range(4):
                emit_g4(NSS - 1, ic)
    nc.finalize()
    return nc


_CACHE = {}


def _get_nc(mode):
    if mode not in _CACHE:
        _CACHE[mode] = build_nc(mode)
    return _CACHE[mode]


def kernel(q, k, v, mask, Wq, bq, Wk, bk, Wv, bv, Wo, bo):
    bf = ml_dtypes.bfloat16
    q = np.asarray(q, np.float32)
    k = np.asarray(k, np.float32)
    v = np.asarray(v, np.float32)
    mask = np.asarray(mask)
    Wq = np.asarray(Wq, np.float32)
    Wk = np.asarray(Wk, np.float32)
    Wv = np.asarray(Wv, np.float32)
    Wo = np.asarray(Wo, np.float32)
    bq = np.asarray(bq, np.float32)
    bk = np.asarray(bk, np.float32)
    bv = np.asarray(bv, np.float32)
    bo = np.asarray(bo, np.float32)

    m = mask.astype(np.float64)
    if np.array_equal(m, np.tril(np.ones((S, S)))):
        mode = "causal"
    elif np.all(m == 1):
        mode = "none"
    else:
        mode = "dense"

    nc = _get_nc(mode)
    tri = np.triu(np.ones((128, 128), np.float32))
    tri2_np = np.concatenate([tri, tri], axis=1).astype(bf)
    id_np = np.eye(128).astype(bf)

    # On-chip layout places local q head h in tile h%4 at partition
    # (h//4)*64 so q/k partition bases match in the scores matmul. Permute
    # Wq columns / Wo rows / bq accordingly: tile cc holds heads (cc, cc+4).
    head_perm = [h for cc in range(4) for h in (cc, cc + 4)]
    col_perm = np.concatenate(
        [np.arange(h * HD, (h + 1) * HD) for h in head_perm])

    # per-batch transposed bf16 inputs (shared across the 4 kv shards)
    qT_b = [np.ascontiguousarray(q[b].astype(bf).T) for b in range(B)]
    kT_b = [np.ascontiguousarray(k[b].astype(bf).T) for b in range(B)]
    vT_b = [np.ascontiguousarray(v[b].astype(bf).T) for b in range(B)]

    in_maps = []
    for core in range(NCORES):
        b, kb = core // KVSH, core % KVSH
        wq_sh = Wq[:, kb * CQ:(kb + 1) * CQ][:, col_perm]
        wo_sh = Wo[kb * CQ:(kb + 1) * CQ, :][col_perm, :]
        bq_sh = bq[kb * CQ:(kb + 1) * CQ][col_perm]
        im = {
            "qT": qT_b[b],
            "kT": kT_b[b],
            "vT": vT_b[b],
            "wq": np.ascontiguousarray(wq_sh.astype(bf)),
            "wk": np.ascontiguousarray(
                Wk[:, kb * CK:(kb + 1) * CK].astype(bf)),
            "wv": np.ascontiguousarray(
                Wv[:, kb * CK:(kb + 1) * CK].astype(bf)),
            "wo": np.ascontiguousarray(wo_sh.astype(bf)),
            "bq": np.ascontiguousarray(bq_sh),
            "bk": np.ascontiguousarray(bk[kb * CK:(kb + 1) * CK]),
            "bv": np.ascontiguousarray(bv[kb * CK:(kb + 1) * CK]),
            "tri2": tri2_np,
            "ident": id_np,
        }
        if mode == "dense":
            with np.errstate(divide="ignore"):
                bias = -(1.0 / mask.astype(np.float32) + 1.0)
            im["mbias"] = np.ascontiguousarray(bias.T * 8.0)
        in_maps.append(im)

    res = run_bass_kernel_spmd(nc, in_maps, core_ids=list(range(NCORES)))
    outs = [r["out"] for r in res.results]
    full = np.empty((B, S, DIM), np.float32)
    for b in range(B):
        acc = outs[b * KVSH].astype(np.float32)
        for kb in range(1, KVSH):
            acc = acc + outs[b * KVSH + kb]
        full[b] = acc + bo[None, :]
    return full


# revision 19
# speedup vs baseline: 1.0416x; 1.0416x over previous
"""Grouped-query attention (GQA) Trainium2 Bass kernel, v2.

Problem: B=2, S=2048, DIM=2048, HQ=32, HKV=8, HEAD_DIM=64, causal mask.
Sharding: 8 cores = 2 (batch) x 4 (kv-head groups). Core c handles batch
c//4 and kv-block c%4 (2 kv heads, 8 q heads). Wq/Wk/Wv sharded
column-wise, Wo row-wise; each core writes a partial [S, DIM] output;
host sums the 4 partials per batch and adds bo.

v2 dataflow (all matmuls bf16 with fp32 PSUM accum):
  - q/k/v are transposed AND cast to bf16 on the HOST -> qT/kT/vT
    [DIM, S] in HBM. No on-chip input transposes or casts; DMA traffic
    halves vs f32 naturals.
  - Weights pre-cast to bf16 on host (columns of Wq / rows of Wo
    permuted so local q-head h sits in tile h%4 at partition (h//4)*64,
    matching its kv head's partition base in kxT).
  - Projections: kxT/vxT first (phase A), then per 512-row i-block:
    qxT, attention, output projection. Biases added on DVE
    (tensor_scalar) during PSUM->SBUF eviction.
  - Scores: the two kv heads of a q-head pair run as row-tiled
    concurrent matmuls (K=64 each, PE row halves 0-63 / 64-127) into
    one 2-bank PSUM tile; ONE Exp activation covers both (3-D AP skips
    the causally-masked tail). Triangular mask applied multiplicatively
    post-exp on diagonal j-blocks only; j-blocks above the diagonal are
    skipped entirely.
  - AV: stationary is [v_head (64 cols) | ones (64 cols)], so PSUM rows
    64:127 accumulate the softmax denominator replicated 64-wide.
    Normalization = full-width DVE reciprocal + 2 multiplies (no DMA
    broadcast, no single-partition ops).
  - Output projection: fp32 partial written straight from a [128, 2048]
    SBUF staging tile, 1 MiB per DMA.
"""

import numpy as np
import ml_dtypes

import concourse.bass as bass
import concourse.mybir as mybir
from concourse import bacc
from concourse.tile import TileContext
from concourse.bass_utils import run_bass_kernel_spmd

# This kernel uses Exp (softmax) and Ln (denominator reciprocal via
# exp(-ln(D))) on the ACT engine. The table-load placement pass maps each
# function to the first table set containing it, which picks
# `exp_and_others` for Exp and `natural_log` for Ln and thrashes
# ACT_TABLE_LOADs (~1.3us + drain each) on every normalization. Both live
# in `natural_log_exp_and_others`; narrow the claimed contents of the
# other sets (names and dict order - hence set ids - are unchanged) so
# the pass settles on the shared set once.
_orig_get_act_tables = bacc.get_activation_tables


def _pinned_act_tables(arch):
    tabs = _orig_get_act_tables(arch)
    exp = mybir.ActivationFunctionType.Exp
    ln = mybir.ActivationFunctionType.Ln
    shared = "natural_log_exp_and_others"
    if shared in tabs and exp in tabs[shared] and ln in tabs[shared]:
        for name, funcs in tabs.items():
            if name != shared:
                tabs[name] = funcs - {exp, ln}
    return tabs


bacc.get_activation_tables = _pinned_act_tables

F32 = mybir.dt.float32
BF16 = mybir.dt.bfloat16
AF = mybir.ActivationFunctionType
ALU = mybir.AluOpType

B, S, DIM = 2, 2048, 2048
HQ, HKV, HD = 32, 8, 64
GROUP = HQ // HKV              # 4
NCORES = 8
KVSH = 4                       # kv-blocks (shards) per batch
CQ = (HQ // KVSH) * HD         # 512 q-proj cols per core (8 heads)
CK = (HKV // KVSH) * HD        # 128 kv-proj cols per core (2 heads)
NDC = DIM // 128               # 16 contraction chunks
NSS = S // 512                 # 4 sequence chunks of 512
NJB = S // 128                 # 16 j-blocks of 128


def _ap3(sl, mid_stride, mid_n, last_n):
    """3-D AP over a 2-D tile slice: [partitions, mid_n x mid_stride,
    last_n] (element strides)."""
    return bass.AP(tensor=sl.tensor, offset=sl.offset,
                   ap=[list(sl.ap[0]), [mid_stride, mid_n], [1, last_n]])


def build_nc(mode="causal"):
    nc = bacc.Bacc("TRN2", target_bir_lowering=False)

    qT = nc.dram_tensor("qT", [DIM, S], BF16, kind="ExternalInput")
    kT = nc.dram_tensor("kT", [DIM, S], BF16, kind="ExternalInput")
    vT = nc.dram_tensor("vT", [DIM, S], BF16, kind="ExternalInput")
    # host pre-gathers weights into SBUF layout: one DMA per tensor
    wq = nc.dram_tensor("wq", [128, NDC * CQ], BF16, kind="ExternalInput")
    wkv = nc.dram_tensor("wkv", [128, NDC * 2 * CK], BF16,
                         kind="ExternalInput")
    wo = nc.dram_tensor("wo", [128, 4 * DIM], BF16, kind="ExternalInput")
    bq = nc.dram_tensor("bq", [CQ], F32, kind="ExternalInput")
    bk = nc.dram_tensor("bk", [CK], F32, kind="ExternalInput")
    bv = nc.dram_tensor("bv", [CK], F32, kind="ExternalInput")
    tri2 = nc.dram_tensor("tri2", [128, 256], BF16, kind="ExternalInput")
    ident = nc.dram_tensor("ident", [128, 128], BF16, kind="ExternalInput")
    mbias = None
    if mode == "dense":
        mbias = nc.dram_tensor("mbias", [S, S], F32, kind="ExternalInput")
    out = nc.dram_tensor("out", [S, DIM], F32, kind="ExternalOutput")

    causal = mode == "causal"

    with TileContext(nc) as tc:
        with (
            tc.tile_pool(name="consts", bufs=1) as consts,
            tc.tile_pool(name="w", bufs=1) as wpool,
            tc.tile_pool(name="qt", bufs=1) as qtp,
            tc.tile_pool(name="stg", bufs=32) as stg,
            tc.tile_pool(name="acts", bufs=1) as acts,
            tc.tile_pool(name="vsb", bufs=2) as vxsb,
            tc.tile_pool(name="exp", bufs=3) as expp,
            tc.tile_pool(name="nm", bufs=2) as nmp,
            tc.tile_pool(name="ob", bufs=2) as obp,
            tc.tile_pool(name="mb", bufs=2) as mbp,
            tc.tile_pool(name="ps_sp", bufs=2, space="PSUM") as ps_sp,
            tc.tile_pool(name="ps_at", bufs=1, space="PSUM") as ps_at,
            tc.tile_pool(name="ps_pj", bufs=2, space="PSUM") as ps_pj,
        ):
            # ---- constants ----
            tri2_t = consts.tile([128, 256], BF16, tag="tri2")
            nc.sync.dma_start(out=tri2_t[:, :], in_=tri2[:, :])
            id_t = consts.tile([128, 128], BF16, tag="id")
            nc.sync.dma_start(out=id_t[:, :], in_=ident[:, :])
            bq_t = consts.tile([128, 4], F32, tag="bq")
            nc.sync.dma_start(
                out=bq_t[:, :],
                in_=bass.AP(tensor=bq[0:1].tensor, offset=0,
                            ap=[[1, 128], [128, 4]]))
            bk_t = consts.tile([128, 1], F32, tag="bk")
            nc.sync.dma_start(
                out=bk_t[:, :],
                in_=bass.AP(tensor=bk[0:1].tensor, offset=0,
                            ap=[[1, 128], [128, 1]]))
            bv_t = consts.tile([128, 1], F32, tag="bv")
            nc.sync.dma_start(
                out=bv_t[:, :],
                in_=bass.AP(tensor=bv[0:1].tensor, offset=0,
                            ap=[[1, 128], [128, 1]]))

            # ---- weights: one DMA each (host pre-gathered layout) ----
            wkv_sb = wpool.tile([128, NDC * 2 * CK], BF16, tag="wkv")
            nc.sync.dma_start(out=wkv_sb[:, :], in_=wkv[:, :])

            def wk_ap(dc):
                return wkv_sb[:, dc * 256:dc * 256 + 128]

            def wv_ap(dc):
                return wkv_sb[:, dc * 256 + 128:(dc + 1) * 256]

            wq_sb = wpool.tile([128, NDC * CQ], BF16, tag="wq")

            def wq_ap(dc, cc):
                return wq_sb[:, dc * CQ + cc * 128:dc * CQ + (cc + 1) * 128]

            wo_sb = wpool.tile([128, 4 * DIM], BF16, tag="wo")

            def wo_ap(cc, ec):
                return wo_sb[:, cc * DIM + ec * 512:cc * DIM + (ec + 1) * 512]

            # ---- qT loads: [128, 1024] per (dc, half). sh=0 rides the
            # SWDGE queue behind vT sh0; sh=1 is emitted at its phase-B
            # use point on HWDGE (slot grants follow emission order, so
            # an up-front emit would deadlock). ----
            qT_t = {}

            def load_qT(sh, eng):
                for dc in range(NDC):
                    t = qtp.tile([128, 1024], BF16, tag=f"qT{dc}",
                                 name=f"qT{dc}_{sh}")
                    eng.dma_start(
                        out=t[:, :],
                        in_=qT[dc * 128:(dc + 1) * 128,
                               sh * 1024:(sh + 1) * 1024])
                    qT_t[(dc, sh)] = t

            # ---- persistent activations ----
            qxT = {}   # (cc, ss) -> [128, 512]; rows 0:64 head cc (kv0),
            #            rows 64:128 head cc+4 (kv1)
            kxT = {}   # ss -> [128, 512]
            attnT = {}  # (pair, ss) -> [128, 512]
            for ss in range(NSS):
                kxT[ss] = acts.tile([128, 512], BF16, tag=f"kx{ss}", name=f"kx{ss}")
                for cc in range(4):
                    qxT[(cc, ss)] = acts.tile([128, 512], BF16,
                                              tag=f"qx{cc}_{ss}", name=f"qx{cc}_{ss}")
                    attnT[(cc, ss)] = acts.tile([128, 512], BF16,
                                                tag=f"at{cc}_{ss}", name=f"at{cc}_{ss}")
            # [v_head | 64 ones cols] per kv head, per j-block
            vx1r = []
            for jb in range(NJB):
                t = acts.tile([128, 256], BF16, tag=f"vp{jb}", name=f"vp{jb}")
                nc.vector.memset(t[:, 64:128], 1.0)
                nc.vector.memset(t[:, 192:256], 1.0)
                vx1r.append(t)

            # ---- phase B helpers ----
            def emit_g1q_cc(ss, cc):
                sh, lo = ss // 2, (ss % 2) * 512
                ps = ps_pj.tile([128, 512], F32, tag="pj")
                for dc in range(NDC):
                    nc.tensor.matmul(
                        ps[:, :], wq_ap(dc, cc),
                        qT_t[(dc, sh)][:, lo:lo + 512],
                        start=(dc == 0), stop=(dc == NDC - 1))
                nc.vector.tensor_scalar_add(qxT[(cc, ss)][:, :],
                                            ps[:, :], bq_t[:, cc:cc + 1])

            def emit_g1q(ss):
                for cc in range(4):
                    emit_g1q_cc(ss, cc)

            def emit_attn_pair(ss, pair):
                s0 = ss * 512
                njb = 4 * (ss + 1) if causal else NJB
                qx = qxT[(pair, ss)]
                at = ps_at.tile([128, 1024], F32, tag="at")
                for jb in range(njb):
                    jss, jr = jb // 4, jb % 4
                    off = max(0, jb * 128 - s0) if causal else 0
                    N = 512 - off
                    sp = ps_sp.tile([128, 1024], F32, tag="sp")
                    nc.tensor.matmul(
                        sp[:, 0:N],
                        kxT[jss][0:64, jr * 128:(jr + 1) * 128],
                        qx[0:64, off:512], start=True, stop=True)
                    nc.tensor.matmul(
                        sp[:, 512:512 + N],
                        kxT[jss][64:128, jr * 128:(jr + 1) * 128],
                        qx[64:128, off:512], start=True, stop=True)
                    if mode == "dense":
                        mb = mbp.tile([128, 512], F32, tag="mb")
                        nc.sync.dma_start(
                            out=mb[:, 0:N],
                            in_=mbias[jb * 128:(jb + 1) * 128,
                                      s0 + off:s0 + 512])
                        nc.vector.tensor_tensor(
                            sp[:, 0:N], sp[:, 0:N], mb[:, 0:N], ALU.add)
                        nc.vector.tensor_tensor(
                            sp[:, 512:512 + N], sp[:, 512:512 + N],
                            mb[:, 0:N], ALU.add)
                    ex = expp.tile([128, 1024], BF16, tag="ex")
                    nc.scalar.activation(
                        _ap3(ex[:, 0:1024], 512, 2, N),
                        _ap3(sp[:, 0:1024], 512, 2, N),
                        AF.Exp, scale=0.125)
                    if causal and jss == ss:
                        nc.vector.tensor_tensor(
                            _ap3(ex[:, 0:1024], 512, 2, 128),
                            _ap3(ex[:, 0:1024], 512, 2, 128),
                            _ap3(tri2_t[:, 0:256], 128, 2, 128),
                            ALU.mult)
                    nc.tensor.matmul(
                        at[:, off:512], vx1r[jb][:, 0:128],
                        ex[:, 0:N],
                        start=(jb == 0), stop=(jb == njb - 1))
                    nc.tensor.matmul(
                        at[:, 512 + off:1024], vx1r[jb][:, 128:256],
                        ex[:, 512:512 + N],
                        start=(jb == 0), stop=(jb == njb - 1))
                # normalize: rows 64:128 hold the denominator replicated
                # 64-wide; 1/D = exp(-ln(D)) on ACT (Exp and Ln share one
                # table set; each call ~6x cheaper than DVE InstReciprocal)
                lnD = nmp.tile([64, 1024], F32, tag="nm")
                nc.scalar.activation(lnD[:, :], at[64:128, 0:1024], AF.Ln)
                nm = nmp.tile([64, 1024], F32, tag="nm")
                nc.scalar.activation(nm[:, :], lnD[:, :], AF.Exp,
                                     scale=-1.0)
                aT = attnT[(pair, ss)]
                nc.vector.tensor_tensor(
                    aT[0:64, :], at[0:64, 0:512], nm[0:64, 0:512],
                    ALU.mult)
                nc.vector.tensor_tensor(
                    aT[64:128, :], at[0:64, 512:1024],
                    nm[0:64, 512:1024], ALU.mult)

            def emit_g4(ss, ic):
                s0 = ss * 512
                i0 = ic * 128
                for hf in range(2):
                    ob = obp.tile([128, 1024], F32, tag="ob", name="ob")
                    for e2 in range(2):
                        ec = hf * 2 + e2
                        g4 = ps_pj.tile([128, 512], F32, tag="pj")
                        for cc2 in range(4):
                            nc.tensor.matmul(
                                g4[:, :],
                                attnT[(cc2, ss)][:, i0:i0 + 128],
                                wo_ap(cc2, ec),
                                start=(cc2 == 0), stop=(cc2 == 3))
                        nc.vector.tensor_copy(
                            ob[:, e2 * 512:(e2 + 1) * 512], g4[:, :])
                    nc.sync.dma_start(
                        out=out[s0 + i0:s0 + i0 + 128,
                                hf * 1024:(hf + 1) * 1024],
                        in_=ob[:, :])

            # ---- phase A: k/v projections + v transpose.
            # Loads split across queues so both stream in parallel:
            # sync(HWDGE): wkv, kT sh0, wq, kT sh1, wo;
            # gpsimd(SWDGE): vT sh0, qT sh0, vT sh1.
            # v-chains run before k-chains (vT lands first). ----
            kvt = {}
            for dc in range(NDC):
                t = stg.tile([128, 1024], BF16, tag="kv", name="kvstg")
                nc.gpsimd.dma_start(
                    out=t[:, :], in_=vT[dc * 128:(dc + 1) * 128, 0:1024])
                kvt[("v", dc, 0)] = t
            for dc in range(NDC):
                t = stg.tile([128, 1024], BF16, tag="kv", name="kvstg")
                nc.sync.dma_start(
                    out=t[:, :], in_=kT[dc * 128:(dc + 1) * 128, 0:1024])
                kvt[("k", dc, 0)] = t
            nc.sync.dma_start(out=wq_sb[:, :], in_=wq[:, :])
            load_qT(0, nc.gpsimd)

            def load_kv_sh1():
                # sh=1 tiles reuse the sh=0 slots, so this is emitted
                # only after the ss0/ss1 chains that release them
                for dc in range(NDC):
                    t = stg.tile([128, 1024], BF16, tag="kv", name="kvstg")
                    nc.sync.dma_start(
                        out=t[:, :],
                        in_=kT[dc * 128:(dc + 1) * 128, 1024:2048])
                    kvt[("k", dc, 1)] = t
                for dc in range(NDC):
                    t = stg.tile([128, 1024], BF16, tag="kv", name="kvstg")
                    nc.gpsimd.dma_start(
                        out=t[:, :],
                        in_=vT[dc * 128:(dc + 1) * 128, 1024:2048])
                    kvt[("v", dc, 1)] = t
                nc.sync.dma_start(out=wo_sb[:, :], in_=wo[:, :])

            for ss in range(NSS):
                if ss == 2:
                    load_kv_sh1()
                sh, lo = ss // 2, (ss % 2) * 512
                ps = ps_pj.tile([128, 512], F32, tag="pj")
                for dc in range(NDC):
                    nc.tensor.matmul(ps[:, :], wv_ap(dc),
                                     kvt[("v", dc, sh)][:, lo:lo + 512],
                                     start=(dc == 0), stop=(dc == NDC - 1))
                vsb = vxsb.tile([128, 512], BF16, tag="vsb")
                nc.vector.tensor_scalar_add(vsb[:, :], ps[:, :],
                                            bv_t[:, 0:1])
                vtp = ps_pj.tile([128, 512], BF16, tag="pj")
                for jr in range(4):
                    nc.tensor.transpose(vtp[:, jr * 128:(jr + 1) * 128],
                                        vsb[:, jr * 128:(jr + 1) * 128],
                                        id_t[:, :])
                for jr in range(4):
                    jb = ss * 4 + jr
                    nc.vector.tensor_copy(
                        vx1r[jb][:, 0:64],
                        vtp[:, jr * 128:jr * 128 + 64])
                    nc.vector.tensor_copy(
                        vx1r[jb][:, 128:192],
                        vtp[:, jr * 128 + 64:jr * 128 + 128])
                ps = ps_pj.tile([128, 512], F32, tag="pj")
                for dc in range(NDC):
                    nc.tensor.matmul(ps[:, :], wk_ap(dc),
                                     kvt[("k", dc, sh)][:, lo:lo + 512],
                                     start=(dc == 0), stop=(dc == NDC - 1))
                nc.vector.tensor_scalar_add(kxT[ss][:, :], ps[:, :],
                                            bk_t[:, 0:1])
                if ss <= 1:
                    # qxT projections for i-blocks 0/1 fill the PE while
                    # the queues stream the remaining k/v tiles
                    emit_g1q(ss)

            # ---- phase B: GEMM4(ss-1) blocks and GEMM1q(ss+1) chains
            # emitted BETWEEN attention pairs of ss: tile-slot grants
            # follow emission order, so this is what lets ready
            # projection matmuls fill the PE idle slots where AV waits
            # on Exp ----
            load_qT(1, nc.sync)
            for ss in range(NSS):
                for pair in range(4):
                    emit_attn_pair(ss, pair)
                    if ss >= 1:
                        emit_g4(ss - 1, pair)
                    if 1 <= ss <= 2:
                        emit_g1q_cc(ss + 1, pair)
            for ic in range(4):
                emit_g4(NSS - 1, ic)
    nc.finalize()
    return nc


_CACHE = {}


def _get_nc(mode):
    if mode not in _CACHE:
        _CACHE[mode] = build_nc(mode)
    return _CACHE[mode]


def kernel(q, k, v, mask, Wq, bq, Wk, bk, Wv, bv, Wo, bo):
    bf = ml_dtypes.bfloat16
    q = np.asarray(q, np.float32)
    k = np.asarray(k, np.float32)
    v = np.asarray(v, np.float32)
    mask = np.asarray(mask)
    Wq = np.asarray(Wq, np.float32)
    Wk = np.asarray(Wk, np.float32)
    Wv = np.asarray(Wv, np.float32)
    Wo = np.asarray(Wo, np.float32)
    bq = np.asarray(bq, np.float32)
    bk = np.asarray(bk, np.float32)
    bv = np.asarray(bv, np.float32)
    bo = np.asarray(bo, np.float32)

    m = mask.astype(np.float64)
    if np.array_equal(m, np.tril(np.ones((S, S)))):
        mode = "causal"
    elif np.all(m == 1):
        mode = "none"
    else:
        mode = "dense"

    nc = _get_nc(mode)
    tri = np.triu(np.ones((128, 128), np.float32))
    tri2_np = np.concatenate([tri, tri], axis=1).astype(bf)
    id_np = np.eye(128).astype(bf)

    # On-chip layout places local q head h in tile h%4 at partition
    # (h//4)*64 so q/k partition bases match in the scores matmul. Permute
    # Wq columns / Wo rows / bq accordingly: tile cc holds heads (cc, cc+4).
    head_perm = [h for cc in range(4) for h in (cc, cc + 4)]
    col_perm = np.concatenate(
        [np.arange(h * HD, (h + 1) * HD) for h in head_perm])

    # per-batch transposed bf16 inputs (shared across the 4 kv shards)
    qT_b = [np.ascontiguousarray(q[b].astype(bf).T) for b in range(B)]
    kT_b = [np.ascontiguousarray(k[b].astype(bf).T) for b in range(B)]
    vT_b = [np.ascontiguousarray(v[b].astype(bf).T) for b in range(B)]

    in_maps = []
    for core in range(NCORES):
        b, kb = core // KVSH, core % KVSH
        wq_sh = Wq[:, kb * CQ:(kb + 1) * CQ][:, col_perm]
        wo_sh = Wo[kb * CQ:(kb + 1) * CQ, :][col_perm, :]
        bq_sh = bq[kb * CQ:(kb + 1) * CQ][col_perm]
        # pre-gather weights into the on-chip layout (dc-major columns)
        wq_g = wq_sh.astype(bf).reshape(NDC, 128, CQ).transpose(1, 0, 2)
        wk_r = Wk[:, kb * CK:(kb + 1) * CK].astype(bf).reshape(NDC, 128, CK)
        wv_r = Wv[:, kb * CK:(kb + 1) * CK].astype(bf).reshape(NDC, 128, CK)
        wkv_g = np.concatenate([wk_r, wv_r], axis=2).transpose(1, 0, 2)
        wo_g = wo_sh.astype(bf).reshape(4, 128, DIM).transpose(1, 0, 2)
        im = {
            "qT": qT_b[b],
            "kT": kT_b[b],
            "vT": vT_b[b],
            "wq": np.ascontiguousarray(wq_g.reshape(128, NDC * CQ)),
            "wkv": np.ascontiguousarray(wkv_g.reshape(128, NDC * 2 * CK)),
            "wo": np.ascontiguousarray(wo_g.reshape(128, 4 * DIM)),
            "bq": np.ascontiguousarray(bq_sh),
            "bk": np.ascontiguousarray(bk[kb * CK:(kb + 1) * CK]),
            "bv": np.ascontiguousarray(bv[kb * CK:(kb + 1) * CK]),
            "tri2": tri2_np,
            "ident": id_np,
        }
        if mode == "dense":
            with np.errstate(divide="ignore"):
                bias = -(1.0 / mask.astype(np.float32) + 1.0)
            im["mbias"] = np.ascontiguousarray(bias.T * 8.0)
        in_maps.append(im)

    res = run_bass_kernel_spmd(nc, in_maps, core_ids=list(range(NCORES)))
    outs = [r["out"] for r in res.results]
    full = np.empty((B, S, DIM), np.float32)
    for b in range(B):
        acc = outs[b * KVSH].astype(np.float32)
        for kb in range(1, KVSH):
            acc = acc + outs[b * KVSH + kb]
        full[b] = acc + bo[None, :]
    return full


# revision 21
# speedup vs baseline: 1.0708x; 1.0280x over previous
"""Grouped-query attention (GQA) Trainium2 Bass kernel, v2.

Problem: B=2, S=2048, DIM=2048, HQ=32, HKV=8, HEAD_DIM=64, causal mask.
Sharding: 8 cores = 2 (batch) x 4 (kv-head groups). Core c handles batch
c//4 and kv-block c%4 (2 kv heads, 8 q heads). Wq/Wk/Wv sharded
column-wise, Wo row-wise; each core writes a partial [S, DIM] output;
host sums the 4 partials per batch and adds bo.

v2 dataflow (all matmuls bf16 with fp32 PSUM accum):
  - q/k/v are transposed AND cast to bf16 on the HOST -> qT/kT/vT
    [DIM, S] in HBM. No on-chip input transposes or casts; DMA traffic
    halves vs f32 naturals.
  - Weights pre-cast to bf16 on host (columns of Wq / rows of Wo
    permuted so local q-head h sits in tile h%4 at partition (h//4)*64,
    matching its kv head's partition base in kxT).
  - Projections: kxT/vxT first (phase A), then per 512-row i-block:
    qxT, attention, output projection. Biases added on DVE
    (tensor_scalar) during PSUM->SBUF eviction.
  - Scores: the two kv heads of a q-head pair run as row-tiled
    concurrent matmuls (K=64 each, PE row halves 0-63 / 64-127) into
    one 2-bank PSUM tile; ONE Exp activation covers both (3-D AP skips
    the causally-masked tail). Triangular mask applied multiplicatively
    post-exp on diagonal j-blocks only; j-blocks above the diagonal are
    skipped entirely.
  - AV: stationary is [v_head (64 cols) | ones (64 cols)], so PSUM rows
    64:127 accumulate the softmax denominator replicated 64-wide.
    Normalization = full-width DVE reciprocal + 2 multiplies (no DMA
    broadcast, no single-partition ops).
  - Output projection: fp32 partial written straight from a [128, 2048]
    SBUF staging tile, 1 MiB per DMA.
"""

import numpy as np
import ml_dtypes

import concourse.bass as bass
import concourse.mybir as mybir
from concourse import bacc
from concourse.tile import TileContext
from concourse.bass_utils import run_bass_kernel_spmd

# This kernel uses Exp (softmax) and Ln (denominator reciprocal via
# exp(-ln(D))) on the ACT engine. The table-load placement pass maps each
# function to the first table set containing it, which picks
# `exp_and_others` for Exp and `natural_log` for Ln and thrashes
# ACT_TABLE_LOADs (~1.3us + drain each) on every normalization. Both live
# in `natural_log_exp_and_others`; narrow the claimed contents of the
# other sets (names and dict order - hence set ids - are unchanged) so
# the pass settles on the shared set once.
_orig_get_act_tables = bacc.get_activation_tables


def _pinned_act_tables(arch):
    tabs = _orig_get_act_tables(arch)
    exp = mybir.ActivationFunctionType.Exp
    ln = mybir.ActivationFunctionType.Ln
    shared = "natural_log_exp_and_others"
    if shared in tabs and exp in tabs[shared] and ln in tabs[shared]:
        for name, funcs in tabs.items():
            if name != shared:
                tabs[name] = funcs - {exp, ln}
    return tabs


bacc.get_activation_tables = _pinned_act_tables

F32 = mybir.dt.float32
BF16 = mybir.dt.bfloat16
AF = mybir.ActivationFunctionType
ALU = mybir.AluOpType

B, S, DIM = 2, 2048, 2048
HQ, HKV, HD = 32, 8, 64
GROUP = HQ // HKV              # 4
NCORES = 8
KVSH = 4                       # kv-blocks (shards) per batch
CQ = (HQ // KVSH) * HD         # 512 q-proj cols per core (8 heads)
CK = (HKV // KVSH) * HD        # 128 kv-proj cols per core (2 heads)
NDC = DIM // 128               # 16 contraction chunks
NSS = S // 512                 # 4 sequence chunks of 512
NJB = S // 128                 # 16 j-blocks of 128


def _ap3(sl, mid_stride, mid_n, last_n):
    """3-D AP over a 2-D tile slice: [partitions, mid_n x mid_stride,
    last_n] (element strides)."""
    return bass.AP(tensor=sl.tensor, offset=sl.offset,
                   ap=[list(sl.ap[0]), [mid_stride, mid_n], [1, last_n]])


def build_nc(mode="causal"):
    nc = bacc.Bacc("TRN2", target_bir_lowering=False)

    qT = nc.dram_tensor("qT", [DIM, S], BF16, kind="ExternalInput")
    kT = nc.dram_tensor("kT", [DIM, S], BF16, kind="ExternalInput")
    vT = nc.dram_tensor("vT", [DIM, S], BF16, kind="ExternalInput")
    # host pre-gathers weights into SBUF layout: one DMA per tensor
    wq = nc.dram_tensor("wq", [128, NDC * CQ], BF16, kind="ExternalInput")
    wkv = nc.dram_tensor("wkv", [128, NDC * 2 * CK], BF16,
                         kind="ExternalInput")
    wo = nc.dram_tensor("wo", [128, 4 * DIM], BF16, kind="ExternalInput")
    bq = nc.dram_tensor("bq", [CQ], F32, kind="ExternalInput")
    bk = nc.dram_tensor("bk", [CK], F32, kind="ExternalInput")
    bv = nc.dram_tensor("bv", [CK], F32, kind="ExternalInput")
    tri2 = nc.dram_tensor("tri2", [128, 256], BF16, kind="ExternalInput")
    ident = nc.dram_tensor("ident", [128, 128], BF16, kind="ExternalInput")
    mbias = None
    if mode == "dense":
        mbias = nc.dram_tensor("mbias", [S, S], F32, kind="ExternalInput")
    out = nc.dram_tensor("out", [S, DIM], F32, kind="ExternalOutput")

    causal = mode == "causal"

    with TileContext(nc) as tc:
        with (
            tc.tile_pool(name="consts", bufs=1) as consts,
            tc.tile_pool(name="w", bufs=1) as wpool,
            tc.tile_pool(name="qt", bufs=1) as qtp,
            tc.tile_pool(name="stg", bufs=32) as stg,
            tc.tile_pool(name="acts", bufs=1) as acts,
            tc.tile_pool(name="vsb", bufs=2) as vxsb,
            tc.tile_pool(name="exp", bufs=3) as expp,
            tc.tile_pool(name="nm", bufs=2) as nmp,
            tc.tile_pool(name="ob", bufs=2) as obp,
            tc.tile_pool(name="mb", bufs=2) as mbp,
            tc.tile_pool(name="ps_sp", bufs=2, space="PSUM") as ps_sp,
            tc.tile_pool(name="ps_at", bufs=1, space="PSUM") as ps_at,
            tc.tile_pool(name="ps_pj", bufs=2, space="PSUM") as ps_pj,
        ):
            # ---- constants ----
            tri2_t = consts.tile([128, 256], BF16, tag="tri2")
            nc.sync.dma_start(out=tri2_t[:, :], in_=tri2[:, :])
            id_t = consts.tile([128, 128], BF16, tag="id")
            nc.sync.dma_start(out=id_t[:, :], in_=ident[:, :])
            bq_t = consts.tile([128, 4], F32, tag="bq")
            nc.sync.dma_start(
                out=bq_t[:, :],
                in_=bass.AP(tensor=bq[0:1].tensor, offset=0,
                            ap=[[1, 128], [128, 4]]))
            bk_t = consts.tile([128, 1], F32, tag="bk")
            nc.sync.dma_start(
                out=bk_t[:, :],
                in_=bass.AP(tensor=bk[0:1].tensor, offset=0,
                            ap=[[1, 128], [128, 1]]))
            bv_t = consts.tile([128, 1], F32, tag="bv")
            nc.sync.dma_start(
                out=bv_t[:, :],
                in_=bass.AP(tensor=bv[0:1].tensor, offset=0,
                            ap=[[1, 128], [128, 1]]))

            # ---- weights: one DMA each (host pre-gathered layout) ----
            wkv_sb = wpool.tile([128, NDC * 2 * CK], BF16, tag="wkv")
            nc.sync.dma_start(out=wkv_sb[:, :], in_=wkv[:, :])

            def wk_ap(dc):
                return wkv_sb[:, dc * 256:dc * 256 + 128]

            def wv_ap(dc):
                return wkv_sb[:, dc * 256 + 128:(dc + 1) * 256]

            wq_sb = wpool.tile([128, NDC * CQ], BF16, tag="wq")

            def wq_ap(dc, cc):
                return wq_sb[:, dc * CQ + cc * 128:dc * CQ + (cc + 1) * 128]

            wo_sb = wpool.tile([128, 4 * DIM], BF16, tag="wo")

            def wo_ap(cc, ec):
                return wo_sb[:, cc * DIM + ec * 512:cc * DIM + (ec + 1) * 512]

            # ---- qT loads: [128, 1024] per (dc, half). sh=0 rides the
            # SWDGE queue behind vT sh0; sh=1 is emitted at its phase-B
            # use point on HWDGE (slot grants follow emission order, so
            # an up-front emit would deadlock). ----
            qT_t = {}

            def load_qT(sh, eng):
                for dc in range(NDC):
                    t = qtp.tile([128, 1024], BF16, tag=f"qT{dc}",
                                 name=f"qT{dc}_{sh}")
                    eng.dma_start(
                        out=t[:, :],
                        in_=qT[dc * 128:(dc + 1) * 128,
                               sh * 1024:(sh + 1) * 1024])
                    qT_t[(dc, sh)] = t

            # ---- persistent activations ----
            qxT = {}   # (cc, ss) -> [128, 512]; rows 0:64 head cc (kv0),
            #            rows 64:128 head cc+4 (kv1)
            kxT = {}   # ss -> [128, 512]
            attnT = {}  # (pair, ss) -> [128, 512]
            for ss in range(NSS):
                kxT[ss] = acts.tile([128, 512], BF16, tag=f"kx{ss}", name=f"kx{ss}")
                for cc in range(4):
                    qxT[(cc, ss)] = acts.tile([128, 512], BF16,
                                              tag=f"qx{cc}_{ss}", name=f"qx{cc}_{ss}")
                    attnT[(cc, ss)] = acts.tile([128, 512], BF16,
                                                tag=f"at{cc}_{ss}", name=f"at{cc}_{ss}")
            # [v_head | 64 ones cols] per kv head, per j-block
            vx1r = []
            for jb in range(NJB):
                t = acts.tile([128, 256], BF16, tag=f"vp{jb}", name=f"vp{jb}")
                nc.vector.memset(t[:, 64:128], 1.0)
                nc.vector.memset(t[:, 192:256], 1.0)
                vx1r.append(t)

            # ---- phase B helpers ----
            def emit_g1q_cc(ss, cc):
                sh, lo = ss // 2, (ss % 2) * 512
                ps = ps_pj.tile([128, 512], F32, tag="pj")
                for dc in range(NDC):
                    nc.tensor.matmul(
                        ps[:, :], wq_ap(dc, cc),
                        qT_t[(dc, sh)][:, lo:lo + 512],
                        start=(dc == 0), stop=(dc == NDC - 1))
                nc.vector.tensor_scalar_add(qxT[(cc, ss)][:, :],
                                            ps[:, :], bq_t[:, cc:cc + 1])

            def emit_g1q(ss):
                for cc in range(4):
                    emit_g1q_cc(ss, cc)

            def emit_attn_pair(ss, pair):
                s0 = ss * 512
                njb = 4 * (ss + 1) if causal else NJB
                qx = qxT[(pair, ss)]
                at = ps_at.tile([128, 1024], F32, tag="at")
                for jb in range(njb):
                    jss, jr = jb // 4, jb % 4
                    off = max(0, jb * 128 - s0) if causal else 0
                    N = 512 - off
                    sp = ps_sp.tile([128, 1024], F32, tag="sp")
                    nc.tensor.matmul(
                        sp[:, 0:N],
                        kxT[jss][0:64, jr * 128:(jr + 1) * 128],
                        qx[0:64, off:512], start=True, stop=True)
                    nc.tensor.matmul(
                        sp[:, 512:512 + N],
                        kxT[jss][64:128, jr * 128:(jr + 1) * 128],
                        qx[64:128, off:512], start=True, stop=True)
                    if mode == "dense":
                        mb = mbp.tile([128, 512], F32, tag="mb")
                        nc.sync.dma_start(
                            out=mb[:, 0:N],
                            in_=mbias[jb * 128:(jb + 1) * 128,
                                      s0 + off:s0 + 512])
                        nc.vector.tensor_tensor(
                            sp[:, 0:N], sp[:, 0:N], mb[:, 0:N], ALU.add)
                        nc.vector.tensor_tensor(
                            sp[:, 512:512 + N], sp[:, 512:512 + N],
                            mb[:, 0:N], ALU.add)
                    ex = expp.tile([128, 1024], BF16, tag="ex")
                    nc.scalar.activation(
                        _ap3(ex[:, 0:1024], 512, 2, N),
                        _ap3(sp[:, 0:1024], 512, 2, N),
                        AF.Exp, scale=0.125)
                    if causal and jss == ss:
                        nc.vector.tensor_tensor(
                            _ap3(ex[:, 0:1024], 512, 2, 128),
                            _ap3(ex[:, 0:1024], 512, 2, 128),
                            _ap3(tri2_t[:, 0:256], 128, 2, 128),
                            ALU.mult)
                    nc.tensor.matmul(
                        at[:, off:512], vx1r[jb][:, 0:128],
                        ex[:, 0:N],
                        start=(jb == 0), stop=(jb == njb - 1))
                    nc.tensor.matmul(
                        at[:, 512 + off:1024], vx1r[jb][:, 128:256],
                        ex[:, 512:512 + N],
                        start=(jb == 0), stop=(jb == njb - 1))
                # normalize: rows 64:128 hold the denominator replicated
                # 64-wide; 1/D = exp(-ln(D)) on ACT (Exp and Ln share one
                # table set; each call ~6x cheaper than DVE InstReciprocal)
                lnD = nmp.tile([64, 1024], F32, tag="nm")
                nc.scalar.activation(lnD[:, :], at[64:128, 0:1024], AF.Ln)
                nm = nmp.tile([64, 1024], F32, tag="nm")
                nc.scalar.activation(nm[:, :], lnD[:, :], AF.Exp,
                                     scale=-1.0)
                aT = attnT[(pair, ss)]
                nc.vector.tensor_tensor(
                    aT[0:64, :], at[0:64, 0:512], nm[0:64, 0:512],
                    ALU.mult)
                nc.vector.tensor_tensor(
                    aT[64:128, :], at[0:64, 512:1024],
                    nm[0:64, 512:1024], ALU.mult)

            def emit_g4(ss, ic):
                s0 = ss * 512
                i0 = ic * 128
                for hf in range(2):
                    ob = obp.tile([128, 1024], F32, tag="ob", name="ob")
                    for e2 in range(2):
                        ec = hf * 2 + e2
                        g4 = ps_pj.tile([128, 512], F32, tag="pj")
                        for cc2 in range(4):
                            nc.tensor.matmul(
                                g4[:, :],
                                attnT[(cc2, ss)][:, i0:i0 + 128],
                                wo_ap(cc2, ec),
                                start=(cc2 == 0), stop=(cc2 == 3))
                        nc.vector.tensor_copy(
                            ob[:, e2 * 512:(e2 + 1) * 512], g4[:, :])
                    nc.sync.dma_start(
                        out=out[s0 + i0:s0 + i0 + 128,
                                hf * 1024:(hf + 1) * 1024],
                        in_=ob[:, :])

            # ---- phase A: k/v projections + v transpose.
            # Loads split across queues so both stream in parallel:
            # sync(HWDGE): wkv, kT sh0, wq, kT sh1, wo;
            # gpsimd(SWDGE): vT sh0, qT sh0, vT sh1.
            # v-chains run before k-chains (vT lands first). ----
            kvt = {}
            for dc in range(NDC):
                t = stg.tile([128, 1024], BF16, tag="kv", name="kvstg")
                nc.gpsimd.dma_start(
                    out=t[:, :], in_=vT[dc * 128:(dc + 1) * 128, 0:1024])
                kvt[("v", dc, 0)] = t
            for dc in range(NDC):
                t = stg.tile([128, 1024], BF16, tag="kv", name="kvstg")
                nc.sync.dma_start(
                    out=t[:, :], in_=kT[dc * 128:(dc + 1) * 128, 0:1024])
                kvt[("k", dc, 0)] = t
            nc.sync.dma_start(out=wq_sb[:, :], in_=wq[:, :])
            load_qT(0, nc.gpsimd)

            def load_kv_sh1():
                # sh=1 tiles reuse the sh=0 slots, so this is emitted
                # only after the ss0/ss1 chains that release them
                for dc in range(NDC):
                    t = stg.tile([128, 1024], BF16, tag="kv", name="kvstg")
                    nc.sync.dma_start(
                        out=t[:, :],
                        in_=kT[dc * 128:(dc + 1) * 128, 1024:2048])
                    kvt[("k", dc, 1)] = t
                for dc in range(NDC):
                    t = stg.tile([128, 1024], BF16, tag="kv", name="kvstg")
                    nc.gpsimd.dma_start(
                        out=t[:, :],
                        in_=vT[dc * 128:(dc + 1) * 128, 1024:2048])
                    kvt[("v", dc, 1)] = t
                nc.sync.dma_start(out=wo_sb[:, :], in_=wo[:, :])

            def emit_kv_chain(ss, which):
                sh, lo = ss // 2, (ss % 2) * 512
                if which == "v":
                    ps = ps_pj.tile([128, 512], F32, tag="pj")
                    for dc in range(NDC):
                        nc.tensor.matmul(ps[:, :], wv_ap(dc),
                                         kvt[("v", dc, sh)][:, lo:lo + 512],
                                         start=(dc == 0),
                                         stop=(dc == NDC - 1))
                    vsb = vxsb.tile([128, 512], BF16, tag="vsb")
                    nc.vector.tensor_scalar_add(vsb[:, :], ps[:, :],
                                                bv_t[:, 0:1])
                    vtp = ps_pj.tile([128, 512], BF16, tag="pj")
                    for jr in range(4):
                        nc.tensor.transpose(
                            vtp[:, jr * 128:(jr + 1) * 128],
                            vsb[:, jr * 128:(jr + 1) * 128], id_t[:, :])
                    for jr in range(4):
                        jb = ss * 4 + jr
                        nc.vector.tensor_copy(
                            vx1r[jb][:, 0:64],
                            vtp[:, jr * 128:jr * 128 + 64])
                        nc.vector.tensor_copy(
                            vx1r[jb][:, 128:192],
                            vtp[:, jr * 128 + 64:jr * 128 + 128])
                else:
                    ps = ps_pj.tile([128, 512], F32, tag="pj")
                    for dc in range(NDC):
                        nc.tensor.matmul(ps[:, :], wk_ap(dc),
                                         kvt[("k", dc, sh)][:, lo:lo + 512],
                                         start=(dc == 0),
                                         stop=(dc == NDC - 1))
                    nc.vector.tensor_scalar_add(kxT[ss][:, :], ps[:, :],
                                                bk_t[:, 0:1])

            # sh=0 chains + qxT(0)/qxT(1) while the queues stream
            for ss in (0, 1):
                emit_kv_chain(ss, "k")
                emit_kv_chain(ss, "v")
                emit_g1q(ss)
            # sh=1 loads: slots freed by the sh=0 chains above
            load_kv_sh1()
            load_qT(1, nc.gpsimd)

            # ---- phase B: attention with projection fills emitted
            # BETWEEN pairs (tile-slot grants follow emission order, so
            # this is what lets ready projection matmuls fill the PE
            # idle slots where AV waits on Exp).
            # ss0 fill: the sh=1 k/v chains; ss1: g4(0)+g1q(2);
            # ss2: g4(1)+g1q(3,0); ss3: g4(2)+staggered g1q(3,*) ----
            for ss in range(NSS):
                for pair in range(4):
                    emit_attn_pair(ss, pair)
                    if ss == 0:
                        emit_kv_chain(2 + pair // 2, "kv"[pair % 2])
                    elif ss == 3:
                        emit_g4(2, pair)
                        if pair < 3:
                            emit_g1q_cc(3, pair + 1)
                    else:
                        emit_g4(ss - 1, pair)
                        if ss == 1:
                            emit_g1q_cc(2, pair)
                        elif pair == 3:
                            emit_g1q_cc(3, 0)
            for ic in range(4):
                emit_g4(NSS - 1, ic)
    nc.finalize()
    return nc


_CACHE = {}


def _get_nc(mode):
    if mode not in _CACHE:
        _CACHE[mode] = build_nc(mode)
    return _CACHE[mode]


def kernel(q, k, v, mask, Wq, bq, Wk, bk, Wv, bv, Wo, bo):
    bf = ml_dtypes.bfloat16
    q = np.asarray(q, np.float32)
    k = np.asarray(k, np.float32)
    v = np.asarray(v, np.float32)
    mask = np.asarray(mask)
    Wq = np.asarray(Wq, np.float32)
    Wk = np.asarray(Wk, np.float32)
    Wv = np.asarray(Wv, np.float32)
    Wo = np.asarray(Wo, np.float32)
    bq = np.asarray(bq, np.float32)
    bk = np.asarray(bk, np.float32)
    bv = np.asarray(bv, np.float32)
    bo = np.asarray(bo, np.float32)

    m = mask.astype(np.float64)
    if np.array_equal(m, np.tril(np.ones((S, S)))):
        mode = "causal"
    elif np.all(m == 1):
        mode = "none"
    else:
        mode = "dense"

    nc = _get_nc(mode)
    tri = np.triu(np.ones((128, 128), np.float32))
    tri2_np = np.concatenate([tri, tri], axis=1).astype(bf)
    id_np = np.eye(128).astype(bf)

    # On-chip layout places local q head h in tile h%4 at partition
    # (h//4)*64 so q/k partition bases match in the scores matmul. Permute
    # Wq columns / Wo rows / bq accordingly: tile cc holds heads (cc, cc+4).
    head_perm = [h for cc in range(4) for h in (cc, cc + 4)]
    col_perm = np.concatenate(
        [np.arange(h * HD, (h + 1) * HD) for h in head_perm])

    # per-batch transposed bf16 inputs (shared across the 4 kv shards)
    qT_b = [np.ascontiguousarray(q[b].astype(bf).T) for b in range(B)]
    kT_b = [np.ascontiguousarray(k[b].astype(bf).T) for b in range(B)]
    vT_b = [np.ascontiguousarray(v[b].astype(bf).T) for b in range(B)]

    in_maps = []
    for core in range(NCORES):
        b, kb = core // KVSH, core % KVSH
        wq_sh = Wq[:, kb * CQ:(kb + 1) * CQ][:, col_perm]
        wo_sh = Wo[kb * CQ:(kb + 1) * CQ, :][col_perm, :]
        bq_sh = bq[kb * CQ:(kb + 1) * CQ][col_perm]
        # pre-gather weights into the on-chip layout (dc-major columns)
        wq_g = wq_sh.astype(bf).reshape(NDC, 128, CQ).transpose(1, 0, 2)
        wk_r = Wk[:, kb * CK:(kb + 1) * CK].astype(bf).reshape(NDC, 128, CK)
        wv_r = Wv[:, kb * CK:(kb + 1) * CK].astype(bf).reshape(NDC, 128, CK)
        wkv_g = np.concatenate([wk_r, wv_r], axis=2).transpose(1, 0, 2)
        wo_g = wo_sh.astype(bf).reshape(4, 128, DIM).transpose(1, 0, 2)
        im = {
            "qT": qT_b[b],
            "kT": kT_b[b],
            "vT": vT_b[b],
            "wq": np.ascontiguousarray(wq_g.reshape(128, NDC * CQ)),
            "wkv": np.ascontiguousarray(wkv_g.reshape(128, NDC * 2 * CK)),
            "wo": np.ascontiguousarray(wo_g.reshape(128, 4 * DIM)),
            "bq": np.ascontiguousarray(bq_sh),
            "bk": np.ascontiguousarray(bk[kb * CK:(kb + 1) * CK]),
            "bv": np.ascontiguousarray(bv[kb * CK:(kb + 1) * CK]),
            "tri2": tri2_np,
            "ident": id_np,
        }
        if mode == "dense":
            with np.errstate(divide="ignore"):
                bias = -(1.0 / mask.astype(np.float32) + 1.0)
            im["mbias"] = np.ascontiguousarray(bias.T * 8.0)
        in_maps.append(im)

    res = run_bass_kernel_spmd(nc, in_maps, core_ids=list(range(NCORES)))
    outs = [r["out"] for r in res.results]
    full = np.empty((B, S, DIM), np.float32)
    for b in range(B):
        acc = outs[b * KVSH].astype(np.float32)
        for kb in range(1, KVSH):
            acc = acc + outs[b * KVSH + kb]
        full[b] = acc + bo[None, :]
    return full


# revision 22
# speedup vs baseline: 1.1368x; 1.0617x over previous
"""Grouped-query attention (GQA) Trainium2 Bass kernel, v2.

Problem: B=2, S=2048, DIM=2048, HQ=32, HKV=8, HEAD_DIM=64, causal mask.
Sharding: 8 cores = 2 (batch) x 4 (kv-head groups). Core c handles batch
c//4 and kv-block c%4 (2 kv heads, 8 q heads). Wq/Wk/Wv sharded
column-wise, Wo row-wise; each core writes a partial [S, DIM] output;
host sums the 4 partials per batch and adds bo.

v2 dataflow (all matmuls bf16 with fp32 PSUM accum):
  - q/k/v are transposed AND cast to bf16 on the HOST -> qT/kT/vT
    [DIM, S] in HBM. No on-chip input transposes or casts; DMA traffic
    halves vs f32 naturals.
  - Weights pre-cast to bf16 on host (columns of Wq / rows of Wo
    permuted so local q-head h sits in tile h%4 at partition (h//4)*64,
    matching its kv head's partition base in kxT).
  - Projections: kxT/vxT first (phase A), then per 512-row i-block:
    qxT, attention, output projection. Biases added on DVE
    (tensor_scalar) during PSUM->SBUF eviction.
  - Scores: the two kv heads of a q-head pair run as row-tiled
    concurrent matmuls (K=64 each, PE row halves 0-63 / 64-127) into
    one 2-bank PSUM tile; ONE Exp activation covers both (3-D AP skips
    the causally-masked tail). Triangular mask applied multiplicatively
    post-exp on diagonal j-blocks only; j-blocks above the diagonal are
    skipped entirely.
  - AV: stationary is [v_head (64 cols) | ones (64 cols)], so PSUM rows
    64:127 accumulate the softmax denominator replicated 64-wide.
    Normalization = full-width DVE reciprocal + 2 multiplies (no DMA
    broadcast, no single-partition ops).
  - Output projection: fp32 partial written straight from a [128, 2048]
    SBUF staging tile, 1 MiB per DMA.
"""

import numpy as np
import ml_dtypes

import concourse.bass as bass
import concourse.mybir as mybir
from concourse import bacc
from concourse.tile import TileContext
from concourse.bass_utils import run_bass_kernel_spmd

# This kernel uses Exp (softmax) and Ln (denominator reciprocal via
# exp(-ln(D))) on the ACT engine. The table-load placement pass maps each
# function to the first table set containing it, which picks
# `exp_and_others` for Exp and `natural_log` for Ln and thrashes
# ACT_TABLE_LOADs (~1.3us + drain each) on every normalization. Both live
# in `natural_log_exp_and_others`; narrow the claimed contents of the
# other sets (names and dict order - hence set ids - are unchanged) so
# the pass settles on the shared set once.
_orig_get_act_tables = bacc.get_activation_tables


def _pinned_act_tables(arch):
    tabs = _orig_get_act_tables(arch)
    exp = mybir.ActivationFunctionType.Exp
    ln = mybir.ActivationFunctionType.Ln
    shared = "natural_log_exp_and_others"
    if shared in tabs and exp in tabs[shared] and ln in tabs[shared]:
        for name, funcs in tabs.items():
            if name != shared:
                tabs[name] = funcs - {exp, ln}
    return tabs


bacc.get_activation_tables = _pinned_act_tables

F32 = mybir.dt.float32
BF16 = mybir.dt.bfloat16
AF = mybir.ActivationFunctionType
ALU = mybir.AluOpType

B, S, DIM = 2, 2048, 2048
HQ, HKV, HD = 32, 8, 64
GROUP = HQ // HKV              # 4
NCORES = 8
KVSH = 4                       # kv-blocks (shards) per batch
CQ = (HQ // KVSH) * HD         # 512 q-proj cols per core (8 heads)
CK = (HKV // KVSH) * HD        # 128 kv-proj cols per core (2 heads)
NDC = DIM // 128               # 16 contraction chunks
NSS = S // 512                 # 4 sequence chunks of 512
NJB = S // 128                 # 16 j-blocks of 128


def _ap3(sl, mid_stride, mid_n, last_n):
    """3-D AP over a 2-D tile slice: [partitions, mid_n x mid_stride,
    last_n] (element strides)."""
    return bass.AP(tensor=sl.tensor, offset=sl.offset,
                   ap=[list(sl.ap[0]), [mid_stride, mid_n], [1, last_n]])


def build_nc(mode="causal"):
    nc = bacc.Bacc("TRN2", target_bir_lowering=False)

    qT = nc.dram_tensor("qT", [DIM, S], BF16, kind="ExternalInput")
    kT = nc.dram_tensor("kT", [DIM, S], BF16, kind="ExternalInput")
    vT = nc.dram_tensor("vT", [DIM, S], BF16, kind="ExternalInput")
    # host pre-gathers weights into SBUF layout: one DMA per tensor
    wq = nc.dram_tensor("wq", [128, NDC * CQ], BF16, kind="ExternalInput")
    wkv = nc.dram_tensor("wkv", [128, NDC * 2 * CK], BF16,
                         kind="ExternalInput")
    wo = nc.dram_tensor("wo", [128, 4 * DIM], BF16, kind="ExternalInput")
    bq = nc.dram_tensor("bq", [CQ], F32, kind="ExternalInput")
    bk = nc.dram_tensor("bk", [CK], F32, kind="ExternalInput")
    bv = nc.dram_tensor("bv", [CK], F32, kind="ExternalInput")
    tri2 = nc.dram_tensor("tri2", [128, 256], BF16, kind="ExternalInput")
    ident = nc.dram_tensor("ident", [128, 128], BF16, kind="ExternalInput")
    mbias = None
    if mode == "dense":
        mbias = nc.dram_tensor("mbias", [S, S], F32, kind="ExternalInput")
    out = nc.dram_tensor("out", [S, DIM], F32, kind="ExternalOutput")

    causal = mode == "causal"

    with TileContext(nc) as tc:
        with (
            tc.tile_pool(name="consts", bufs=1) as consts,
            tc.tile_pool(name="w", bufs=1) as wpool,
            tc.tile_pool(name="qt", bufs=1) as qtp,
            tc.tile_pool(name="stg", bufs=32) as stg,
            tc.tile_pool(name="acts", bufs=1) as acts,
            tc.tile_pool(name="vsb", bufs=2) as vxsb,
            tc.tile_pool(name="exp", bufs=3) as expp,
            tc.tile_pool(name="nm", bufs=2) as nmp,
            tc.tile_pool(name="ob", bufs=2) as obp,
            tc.tile_pool(name="mb", bufs=2) as mbp,
            tc.tile_pool(name="ps_sp", bufs=2, space="PSUM") as ps_sp,
            tc.tile_pool(name="ps_at", bufs=1, space="PSUM") as ps_at,
            tc.tile_pool(name="ps_pj", bufs=2, space="PSUM") as ps_pj,
        ):
            # ---- constants ----
            tri2_t = consts.tile([128, 256], BF16, tag="tri2")
            nc.sync.dma_start(out=tri2_t[:, :], in_=tri2[:, :])
            id_t = consts.tile([128, 128], BF16, tag="id")
            nc.sync.dma_start(out=id_t[:, :], in_=ident[:, :])
            bq_t = consts.tile([128, 4], F32, tag="bq")
            nc.sync.dma_start(
                out=bq_t[:, :],
                in_=bass.AP(tensor=bq[0:1].tensor, offset=0,
                            ap=[[1, 128], [128, 4]]))
            bk_t = consts.tile([128, 1], F32, tag="bk")
            nc.sync.dma_start(
                out=bk_t[:, :],
                in_=bass.AP(tensor=bk[0:1].tensor, offset=0,
                            ap=[[1, 128], [128, 1]]))
            bv_t = consts.tile([128, 1], F32, tag="bv")
            nc.sync.dma_start(
                out=bv_t[:, :],
                in_=bass.AP(tensor=bv[0:1].tensor, offset=0,
                            ap=[[1, 128], [128, 1]]))

            # ---- weights: one DMA each (host pre-gathered layout) ----
            wkv_sb = wpool.tile([128, NDC * 2 * CK], BF16, tag="wkv")
            nc.sync.dma_start(out=wkv_sb[:, :], in_=wkv[:, :])

            def wk_ap(dc):
                return wkv_sb[:, dc * 256:dc * 256 + 128]

            def wv_ap(dc):
                return wkv_sb[:, dc * 256 + 128:(dc + 1) * 256]

            wq_sb = wpool.tile([128, NDC * CQ], BF16, tag="wq")

            def wq_ap(dc, cc):
                return wq_sb[:, dc * CQ + cc * 128:dc * CQ + (cc + 1) * 128]

            wo_sb = wpool.tile([128, 4 * DIM], BF16, tag="wo")

            def wo_ap(cc, ec):
                return wo_sb[:, cc * DIM + ec * 512:cc * DIM + (ec + 1) * 512]

            # ---- qT loads: [128, 1024] per (dc, half). sh=0 rides the
            # SWDGE queue behind vT sh0; sh=1 is emitted at its phase-B
            # use point on HWDGE (slot grants follow emission order, so
            # an up-front emit would deadlock). ----
            qT_t = {}

            def load_qT(sh, eng):
                for dc in range(NDC):
                    t = qtp.tile([128, 1024], BF16, tag=f"qT{dc}",
                                 name=f"qT{dc}_{sh}")
                    eng.dma_start(
                        out=t[:, :],
                        in_=qT[dc * 128:(dc + 1) * 128,
                               sh * 1024:(sh + 1) * 1024])
                    qT_t[(dc, sh)] = t

            # ---- persistent activations ----
            qxT = {}   # (cc, ss) -> [128, 512]; rows 0:64 head cc (kv0),
            #            rows 64:128 head cc+4 (kv1)
            kxT = {}   # ss -> [128, 512]
            attnT = {}  # (pair, ss) -> [128, 512]
            for ss in range(NSS):
                kxT[ss] = acts.tile([128, 512], BF16, tag=f"kx{ss}", name=f"kx{ss}")
                for cc in range(4):
                    qxT[(cc, ss)] = acts.tile([128, 512], BF16,
                                              tag=f"qx{cc}_{ss}", name=f"qx{cc}_{ss}")
                    attnT[(cc, ss)] = acts.tile([128, 512], BF16,
                                                tag=f"at{cc}_{ss}", name=f"at{cc}_{ss}")
            # [v_head | 64 ones cols] per kv head, per j-block
            vx1r = []
            for jb in range(NJB):
                t = acts.tile([128, 256], BF16, tag=f"vp{jb}", name=f"vp{jb}")
                nc.vector.memset(t[:, 64:128], 1.0)
                nc.vector.memset(t[:, 192:256], 1.0)
                vx1r.append(t)

            # ---- phase B helpers ----
            def emit_g1q_cc(ss, cc):
                sh, lo = ss // 2, (ss % 2) * 512
                ps = ps_pj.tile([128, 512], F32, tag="pj")
                for dc in range(NDC):
                    nc.tensor.matmul(
                        ps[:, :], wq_ap(dc, cc),
                        qT_t[(dc, sh)][:, lo:lo + 512],
                        start=(dc == 0), stop=(dc == NDC - 1))
                nc.vector.tensor_scalar_add(qxT[(cc, ss)][:, :],
                                            ps[:, :], bq_t[:, cc:cc + 1])

            def emit_g1q(ss):
                for cc in range(4):
                    emit_g1q_cc(ss, cc)

            def emit_attn_pair(ss, pair):
                s0 = ss * 512
                njb = 4 * (ss + 1) if causal else NJB
                qx = qxT[(pair, ss)]
                at = ps_at.tile([128, 1024], F32, tag="at")
                for jb in range(njb):
                    jss, jr = jb // 4, jb % 4
                    off = max(0, jb * 128 - s0) if causal else 0
                    N = 512 - off
                    sp = ps_sp.tile([128, 1024], F32, tag="sp")
                    nc.tensor.matmul(
                        sp[:, 0:N],
                        kxT[jss][0:64, jr * 128:(jr + 1) * 128],
                        qx[0:64, off:512], start=True, stop=True)
                    nc.tensor.matmul(
                        sp[:, 512:512 + N],
                        kxT[jss][64:128, jr * 128:(jr + 1) * 128],
                        qx[64:128, off:512], start=True, stop=True)
                    if mode == "dense":
                        mb = mbp.tile([128, 512], F32, tag="mb")
                        nc.sync.dma_start(
                            out=mb[:, 0:N],
                            in_=mbias[jb * 128:(jb + 1) * 128,
                                      s0 + off:s0 + 512])
                        nc.vector.tensor_tensor(
                            sp[:, 0:N], sp[:, 0:N], mb[:, 0:N], ALU.add)
                        nc.vector.tensor_tensor(
                            sp[:, 512:512 + N], sp[:, 512:512 + N],
                            mb[:, 0:N], ALU.add)
                    ex = expp.tile([128, 1024], BF16, tag="ex")
                    nc.scalar.activation(
                        _ap3(ex[:, 0:1024], 512, 2, N),
                        _ap3(sp[:, 0:1024], 512, 2, N),
                        AF.Exp, scale=0.125)
                    if causal and jss == ss:
                        nc.vector.tensor_tensor(
                            _ap3(ex[:, 0:1024], 512, 2, 128),
                            _ap3(ex[:, 0:1024], 512, 2, 128),
                            _ap3(tri2_t[:, 0:256], 128, 2, 128),
                            ALU.mult)
                    nc.tensor.matmul(
                        at[:, off:512], vx1r[jb][:, 0:128],
                        ex[:, 0:N],
                        start=(jb == 0), stop=(jb == njb - 1))
                    nc.tensor.matmul(
                        at[:, 512 + off:1024], vx1r[jb][:, 128:256],
                        ex[:, 512:512 + N],
                        start=(jb == 0), stop=(jb == njb - 1))
                # normalize: rows 64:128 hold the denominator replicated
                # 64-wide; 1/D = exp(-ln(D)) on ACT (Exp and Ln share one
                # table set; each call ~6x cheaper than DVE InstReciprocal)
                lnD = nmp.tile([64, 1024], F32, tag="nm")
                nc.scalar.activation(lnD[:, :], at[64:128, 0:1024], AF.Ln)
                nm = nmp.tile([64, 1024], F32, tag="nm")
                nc.scalar.activation(nm[:, :], lnD[:, :], AF.Exp,
                                     scale=-1.0)
                aT = attnT[(pair, ss)]
                nc.vector.tensor_tensor(
                    aT[0:64, :], at[0:64, 0:512], nm[0:64, 0:512],
                    ALU.mult)
                nc.vector.tensor_tensor(
                    aT[64:128, :], at[0:64, 512:1024],
                    nm[0:64, 512:1024], ALU.mult)

            def emit_g4(ss, ic):
                s0 = ss * 512
                i0 = ic * 128
                for hf in range(2):
                    ob = obp.tile([128, 1024], F32, tag="ob", name="ob")
                    for e2 in range(2):
                        ec = hf * 2 + e2
                        g4 = ps_pj.tile([128, 512], F32, tag="pj")
                        for cc2 in range(4):
                            nc.tensor.matmul(
                                g4[:, :],
                                attnT[(cc2, ss)][:, i0:i0 + 128],
                                wo_ap(cc2, ec),
                                start=(cc2 == 0), stop=(cc2 == 3))
                        nc.vector.tensor_copy(
                            ob[:, e2 * 512:(e2 + 1) * 512], g4[:, :])
                    nc.sync.dma_start(
                        out=out[s0 + i0:s0 + i0 + 128,
                                hf * 1024:(hf + 1) * 1024],
                        in_=ob[:, :])

            # ---- phase A: k/v projections + v transpose.
            # Loads split across queues so both stream in parallel:
            # sync(HWDGE): wkv, kT sh0, wq, kT sh1, wo;
            # gpsimd(SWDGE): vT sh0, qT sh0, vT sh1.
            # v-chains run before k-chains (vT lands first). ----
            kvt = {}
            for dc in range(NDC):
                t = stg.tile([128, 1024], BF16, tag="kv", name="kvstg")
                nc.gpsimd.dma_start(
                    out=t[:, :], in_=vT[dc * 128:(dc + 1) * 128, 0:1024])
                kvt[("v", dc, 0)] = t
            for dc in range(NDC):
                t = stg.tile([128, 1024], BF16, tag="kv", name="kvstg")
                nc.sync.dma_start(
                    out=t[:, :], in_=kT[dc * 128:(dc + 1) * 128, 0:1024])
                kvt[("k", dc, 0)] = t
            nc.sync.dma_start(out=wq_sb[:, :], in_=wq[:, :])
            load_qT(0, nc.gpsimd)

            def load_kv_sh1():
                # sh=1 tiles reuse the sh=0 slots, so this is emitted
                # only after the ss0/ss1 chains that release them
                for dc in range(NDC):
                    t = stg.tile([128, 1024], BF16, tag="kv", name="kvstg")
                    nc.sync.dma_start(
                        out=t[:, :],
                        in_=kT[dc * 128:(dc + 1) * 128, 1024:2048])
                    kvt[("k", dc, 1)] = t
                for dc in range(NDC):
                    t = stg.tile([128, 1024], BF16, tag="kv", name="kvstg")
                    nc.gpsimd.dma_start(
                        out=t[:, :],
                        in_=vT[dc * 128:(dc + 1) * 128, 1024:2048])
                    kvt[("v", dc, 1)] = t
                nc.sync.dma_start(out=wo_sb[:, :], in_=wo[:, :])

            def emit_kv_chain(ss, which):
                sh, lo = ss // 2, (ss % 2) * 512
                if which == "v":
                    ps = ps_pj.tile([128, 512], F32, tag="pj")
                    for dc in range(NDC):
                        nc.tensor.matmul(ps[:, :], wv_ap(dc),
                                         kvt[("v", dc, sh)][:, lo:lo + 512],
                                         start=(dc == 0),
                                         stop=(dc == NDC - 1))
                    vsb = vxsb.tile([128, 512], BF16, tag="vsb")
                    nc.vector.tensor_scalar_add(vsb[:, :], ps[:, :],
                                                bv_t[:, 0:1])
                    vtp = ps_pj.tile([128, 512], BF16, tag="pj")
                    for jr in range(4):
                        nc.tensor.transpose(
                            vtp[:, jr * 128:(jr + 1) * 128],
                            vsb[:, jr * 128:(jr + 1) * 128], id_t[:, :])
                    for jr in range(4):
                        jb = ss * 4 + jr
                        nc.vector.tensor_copy(
                            vx1r[jb][:, 0:64],
                            vtp[:, jr * 128:jr * 128 + 64])
                        nc.vector.tensor_copy(
                            vx1r[jb][:, 128:192],
                            vtp[:, jr * 128 + 64:jr * 128 + 128])
                else:
                    ps = ps_pj.tile([128, 512], F32, tag="pj")
                    for dc in range(NDC):
                        nc.tensor.matmul(ps[:, :], wk_ap(dc),
                                         kvt[("k", dc, sh)][:, lo:lo + 512],
                                         start=(dc == 0),
                                         stop=(dc == NDC - 1))
                    nc.vector.tensor_scalar_add(kxT[ss][:, :], ps[:, :],
                                                bk_t[:, 0:1])

            # ---- driver: the ACT exp/norm stream is the serialized
            # critical path (~230us), so attention pairs start as early
            # as possible (just-in-time qxT chains) and ALL other PE
            # work -- k/v chains, GEMM4, second-half loads -- is emitted
            # between pairs as fill. Tile-slot grants follow emission
            # order, so fills sit next to attention in priority and get
            # pulled into the PE idle slots where AV waits on Exp. ----
            emit_kv_chain(0, "k")
            emit_kv_chain(0, "v")
            for ss in range(NSS):
                for pair in range(4):
                    emit_g1q_cc(ss, pair)
                    emit_attn_pair(ss, pair)
                    if ss == 0:
                        if pair == 0:
                            emit_kv_chain(1, "k")
                        elif pair == 1:
                            emit_kv_chain(1, "v")
                        elif pair == 2:
                            load_kv_sh1()
                            load_qT(1, nc.gpsimd)
                    elif ss == 1:
                        emit_g4(0, pair)
                        emit_kv_chain(2 + pair // 2, "kv"[pair % 2])
                    else:
                        emit_g4(ss - 1, pair)
            for ic in range(4):
                emit_g4(NSS - 1, ic)
    nc.finalize()
    return nc


_CACHE = {}


def _get_nc(mode):
    if mode not in _CACHE:
        _CACHE[mode] = build_nc(mode)
    return _CACHE[mode]


def kernel(q, k, v, mask, Wq, bq, Wk, bk, Wv, bv, Wo, bo):
    bf = ml_dtypes.bfloat16
    q = np.asarray(q, np.float32)
    k = np.asarray(k, np.float32)
    v = np.asarray(v, np.float32)
    mask = np.asarray(mask)
    Wq = np.asarray(Wq, np.float32)
    Wk = np.asarray(Wk, np.float32)
    Wv = np.asarray(Wv, np.float32)
    Wo = np.asarray(Wo, np.float32)
    bq = np.asarray(bq, np.float32)
    bk = np.asarray(bk, np.float32)
    bv = np.asarray(bv, np.float32)
    bo = np.asarray(bo, np.float32)

    m = mask.astype(np.float64)
    if np.array_equal(m, np.tril(np.ones((S, S)))):
        mode = "causal"
    elif np.all(m == 1):
        mode = "none"
    else:
        mode = "dense"

    nc = _get_nc(mode)
    tri = np.triu(np.ones((128, 128), np.float32))
    tri2_np = np.concatenate([tri, tri], axis=1).astype(bf)
    id_np = np.eye(128).astype(bf)

    # On-chip layout places local q head h in tile h%4 at partition
    # (h//4)*64 so q/k partition bases match in the scores matmul. Permute
    # Wq columns / Wo rows / bq accordingly: tile cc holds heads (cc, cc+4).
    head_perm = [h for cc in range(4) for h in (cc, cc + 4)]
    col_perm = np.concatenate(
        [np.arange(h * HD, (h + 1) * HD) for h in head_perm])

    # per-batch transposed bf16 inputs (shared across the 4 kv shards)
    qT_b = [np.ascontiguousarray(q[b].astype(bf).T) for b in range(B)]
    kT_b = [np.ascontiguousarray(k[b].astype(bf).T) for b in range(B)]
    vT_b = [np.ascontiguousarray(v[b].astype(bf).T) for b in range(B)]

    in_maps = []
    for core in range(NCORES):
        b, kb = core // KVSH, core % KVSH
        wq_sh = Wq[:, kb * CQ:(kb + 1) * CQ][:, col_perm]
        wo_sh = Wo[kb * CQ:(kb + 1) * CQ, :][col_perm, :]
        bq_sh = bq[kb * CQ:(kb + 1) * CQ][col_perm]
        # pre-gather weights into the on-chip layout (dc-major columns)
        wq_g = wq_sh.astype(bf).reshape(NDC, 128, CQ).transpose(1, 0, 2)
        wk_r = Wk[:, kb * CK:(kb + 1) * CK].astype(bf).reshape(NDC, 128, CK)
        wv_r = Wv[:, kb * CK:(kb + 1) * CK].astype(bf).reshape(NDC, 128, CK)
        wkv_g = np.concatenate([wk_r, wv_r], axis=2).transpose(1, 0, 2)
        wo_g = wo_sh.astype(bf).reshape(4, 128, DIM).transpose(1, 0, 2)
        im = {
            "qT": qT_b[b],
            "kT": kT_b[b],
            "vT": vT_b[b],
            "wq": np.ascontiguousarray(wq_g.reshape(128, NDC * CQ)),
            "wkv": np.ascontiguousarray(wkv_g.reshape(128, NDC * 2 * CK)),
            "wo": np.ascontiguousarray(wo_g.reshape(128, 4 * DIM)),
            "bq": np.ascontiguousarray(bq_sh),
            "bk": np.ascontiguousarray(bk[kb * CK:(kb + 1) * CK]),
            "bv": np.ascontiguousarray(bv[kb * CK:(kb + 1) * CK]),
            "tri2": tri2_np,
            "ident": id_np,
        }
        if mode == "dense":
            with np.errstate(divide="ignore"):
                bias = -(1.0 / mask.astype(np.float32) + 1.0)
            im["mbias"] = np.ascontiguousarray(bias.T * 8.0)
        in_maps.append(im)

    res = run_bass_kernel_spmd(nc, in_maps, core_ids=list(range(NCORES)))
    outs = [r["out"] for r in res.results]
    full = np.empty((B, S, DIM), np.float32)
    for b in range(B):
        acc = outs[b * KVSH].astype(np.float32)
        for kb in range(1, KVSH):
            acc = acc + outs[b * KVSH + kb]
        full[b] = acc + bo[None, :]
    return full


# revision 23
# speedup vs baseline: 1.1390x; 1.0019x over previous
"""Grouped-query attention (GQA) Trainium2 Bass kernel, v2.

Problem: B=2, S=2048, DIM=2048, HQ=32, HKV=8, HEAD_DIM=64, causal mask.
Sharding: 8 cores = 2 (batch) x 4 (kv-head groups). Core c handles batch
c//4 and kv-block c%4 (2 kv heads, 8 q heads). Wq/Wk/Wv sharded
column-wise, Wo row-wise; each core writes a partial [S, DIM] output;
host sums the 4 partials per batch and adds bo.

v2 dataflow (all matmuls bf16 with fp32 PSUM accum):
  - q/k/v are transposed AND cast to bf16 on the HOST -> qT/kT/vT
    [DIM, S] in HBM. No on-chip input transposes or casts; DMA traffic
    halves vs f32 naturals.
  - Weights pre-cast to bf16 on host (columns of Wq / rows of Wo
    permuted so local q-head h sits in tile h%4 at partition (h//4)*64,
    matching its kv head's partition base in kxT).
  - Projections: kxT/vxT first (phase A), then per 512-row i-block:
    qxT, attention, output projection. Biases added on DVE
    (tensor_scalar) during PSUM->SBUF eviction.
  - Scores: the two kv heads of a q-head pair run as row-tiled
    concurrent matmuls (K=64 each, PE row halves 0-63 / 64-127) into
    one 2-bank PSUM tile; ONE Exp activation covers both (3-D AP skips
    the causally-masked tail). Triangular mask applied multiplicatively
    post-exp on diagonal j-blocks only; j-blocks above the diagonal are
    skipped entirely.
  - AV: stationary is [v_head (64 cols) | ones (64 cols)], so PSUM rows
    64:127 accumulate the softmax denominator replicated 64-wide.
    Normalization = full-width DVE reciprocal + 2 multiplies (no DMA
    broadcast, no single-partition ops).
  - Output projection: fp32 partial written straight from a [128, 2048]
    SBUF staging tile, 1 MiB per DMA.
"""

import numpy as np
import ml_dtypes

import concourse.bass as bass
import concourse.mybir as mybir
from concourse import bacc
from concourse.tile import TileContext
from concourse.bass_utils import run_bass_kernel_spmd

# This kernel uses Exp (softmax) and Ln (denominator reciprocal via
# exp(-ln(D))) on the ACT engine. The table-load placement pass maps each
# function to the first table set containing it, which picks
# `exp_and_others` for Exp and `natural_log` for Ln and thrashes
# ACT_TABLE_LOADs (~1.3us + drain each) on every normalization. Both live
# in `natural_log_exp_and_others`; narrow the claimed contents of the
# other sets (names and dict order - hence set ids - are unchanged) so
# the pass settles on the shared set once.
_orig_get_act_tables = bacc.get_activation_tables


def _pinned_act_tables(arch):
    tabs = _orig_get_act_tables(arch)
    exp = mybir.ActivationFunctionType.Exp
    ln = mybir.ActivationFunctionType.Ln
    shared = "natural_log_exp_and_others"
    if shared in tabs and exp in tabs[shared] and ln in tabs[shared]:
        for name, funcs in tabs.items():
            if name != shared:
                tabs[name] = funcs - {exp, ln}
    return tabs


bacc.get_activation_tables = _pinned_act_tables

F32 = mybir.dt.float32
BF16 = mybir.dt.bfloat16
AF = mybir.ActivationFunctionType
ALU = mybir.AluOpType

B, S, DIM = 2, 2048, 2048
HQ, HKV, HD = 32, 8, 64
GROUP = HQ // HKV              # 4
NCORES = 8
KVSH = 4                       # kv-blocks (shards) per batch
CQ = (HQ // KVSH) * HD         # 512 q-proj cols per core (8 heads)
CK = (HKV // KVSH) * HD        # 128 kv-proj cols per core (2 heads)
NDC = DIM // 128               # 16 contraction chunks
NSS = S // 512                 # 4 sequence chunks of 512
NJB = S // 128                 # 16 j-blocks of 128


def _ap3(sl, mid_stride, mid_n, last_n):
    """3-D AP over a 2-D tile slice: [partitions, mid_n x mid_stride,
    last_n] (element strides)."""
    return bass.AP(tensor=sl.tensor, offset=sl.offset,
                   ap=[list(sl.ap[0]), [mid_stride, mid_n], [1, last_n]])


def build_nc(mode="causal"):
    nc = bacc.Bacc("TRN2", target_bir_lowering=False)

    qT = nc.dram_tensor("qT", [DIM, S], BF16, kind="ExternalInput")
    kT = nc.dram_tensor("kT", [DIM, S], BF16, kind="ExternalInput")
    vT = nc.dram_tensor("vT", [DIM, S], BF16, kind="ExternalInput")
    # host pre-gathers weights into SBUF layout: one DMA per tensor
    wq = nc.dram_tensor("wq", [128, NDC * CQ], BF16, kind="ExternalInput")
    wkv = nc.dram_tensor("wkv", [128, NDC * 2 * CK], BF16,
                         kind="ExternalInput")
    wo = nc.dram_tensor("wo", [128, 4 * DIM], BF16, kind="ExternalInput")
    bq = nc.dram_tensor("bq", [CQ], F32, kind="ExternalInput")
    bk = nc.dram_tensor("bk", [CK], F32, kind="ExternalInput")
    bv = nc.dram_tensor("bv", [CK], F32, kind="ExternalInput")
    tri2 = nc.dram_tensor("tri2", [128, 256], BF16, kind="ExternalInput")
    ident = nc.dram_tensor("ident", [128, 128], BF16, kind="ExternalInput")
    mbias = None
    if mode == "dense":
        mbias = nc.dram_tensor("mbias", [S, S], F32, kind="ExternalInput")
    out = nc.dram_tensor("out", [S, DIM], F32, kind="ExternalOutput")

    causal = mode == "causal"

    with TileContext(nc) as tc:
        with (
            tc.tile_pool(name="consts", bufs=1) as consts,
            tc.tile_pool(name="w", bufs=1) as wpool,
            tc.tile_pool(name="qt", bufs=1) as qtp,
            tc.tile_pool(name="stg", bufs=32) as stg,
            tc.tile_pool(name="acts", bufs=1) as acts,
            tc.tile_pool(name="vsb", bufs=2) as vxsb,
            tc.tile_pool(name="exp", bufs=4) as expp,
            tc.tile_pool(name="nm", bufs=2) as nmp,
            tc.tile_pool(name="ob", bufs=2) as obp,
            tc.tile_pool(name="mb", bufs=2) as mbp,
            tc.tile_pool(name="ps_sp", bufs=2, space="PSUM") as ps_sp,
            tc.tile_pool(name="ps_at", bufs=1, space="PSUM") as ps_at,
            tc.tile_pool(name="ps_pj", bufs=2, space="PSUM") as ps_pj,
        ):
            # ---- constants ----
            tri2_t = consts.tile([128, 256], BF16, tag="tri2")
            nc.sync.dma_start(out=tri2_t[:, :], in_=tri2[:, :])
            id_t = consts.tile([128, 128], BF16, tag="id")
            nc.sync.dma_start(out=id_t[:, :], in_=ident[:, :])
            bq_t = consts.tile([128, 4], F32, tag="bq")
            nc.sync.dma_start(
                out=bq_t[:, :],
                in_=bass.AP(tensor=bq[0:1].tensor, offset=0,
                            ap=[[1, 128], [128, 4]]))
            bk_t = consts.tile([128, 1], F32, tag="bk")
            nc.sync.dma_start(
                out=bk_t[:, :],
                in_=bass.AP(tensor=bk[0:1].tensor, offset=0,
                            ap=[[1, 128], [128, 1]]))
            bv_t = consts.tile([128, 1], F32, tag="bv")
            nc.sync.dma_start(
                out=bv_t[:, :],
                in_=bass.AP(tensor=bv[0:1].tensor, offset=0,
                            ap=[[1, 128], [128, 1]]))

            # ---- weights: one DMA each (host pre-gathered layout) ----
            wkv_sb = wpool.tile([128, NDC * 2 * CK], BF16, tag="wkv")
            nc.sync.dma_start(out=wkv_sb[:, :], in_=wkv[:, :])

            def wk_ap(dc):
                return wkv_sb[:, dc * 256:dc * 256 + 128]

            def wv_ap(dc):
                return wkv_sb[:, dc * 256 + 128:(dc + 1) * 256]

            wq_sb = wpool.tile([128, NDC * CQ], BF16, tag="wq")

            def wq_ap(dc, cc):
                return wq_sb[:, dc * CQ + cc * 128:dc * CQ + (cc + 1) * 128]

            wo_sb = wpool.tile([128, 4 * DIM], BF16, tag="wo")

            def wo_ap(cc, ec):
                return wo_sb[:, cc * DIM + ec * 512:cc * DIM + (ec + 1) * 512]

            # ---- qT loads: [128, 1024] per (dc, half). sh=0 rides the
            # SWDGE queue behind vT sh0; sh=1 is emitted at its phase-B
            # use point on HWDGE (slot grants follow emission order, so
            # an up-front emit would deadlock). ----
            qT_t = {}

            def load_qT(sh, eng):
                for dc in range(NDC):
                    t = qtp.tile([128, 1024], BF16, tag=f"qT{dc}",
                                 name=f"qT{dc}_{sh}")
                    eng.dma_start(
                        out=t[:, :],
                        in_=qT[dc * 128:(dc + 1) * 128,
                               sh * 1024:(sh + 1) * 1024])
                    qT_t[(dc, sh)] = t

            # ---- persistent activations ----
            qxT = {}   # (cc, ss) -> [128, 512]; rows 0:64 head cc (kv0),
            #            rows 64:128 head cc+4 (kv1)
            kxT = {}   # ss -> [128, 512]
            attnT = {}  # (pair, ss) -> [128, 512]
            for ss in range(NSS):
                kxT[ss] = acts.tile([128, 512], BF16, tag=f"kx{ss}", name=f"kx{ss}")
                for cc in range(4):
                    qxT[(cc, ss)] = acts.tile([128, 512], BF16,
                                              tag=f"qx{cc}_{ss}", name=f"qx{cc}_{ss}")
                    attnT[(cc, ss)] = acts.tile([128, 512], BF16,
                                                tag=f"at{cc}_{ss}", name=f"at{cc}_{ss}")
            # [v_head | 64 ones cols] per kv head, per j-block
            vx1r = []
            for jb in range(NJB):
                t = acts.tile([128, 256], BF16, tag=f"vp{jb}", name=f"vp{jb}")
                nc.vector.memset(t[:, 64:128], 1.0)
                nc.vector.memset(t[:, 192:256], 1.0)
                vx1r.append(t)

            # ---- phase B helpers ----
            def emit_g1q_cc(ss, cc):
                sh, lo = ss // 2, (ss % 2) * 512
                ps = ps_pj.tile([128, 512], F32, tag="pj")
                for dc in range(NDC):
                    nc.tensor.matmul(
                        ps[:, :], wq_ap(dc, cc),
                        qT_t[(dc, sh)][:, lo:lo + 512],
                        start=(dc == 0), stop=(dc == NDC - 1))
                nc.vector.tensor_scalar_add(qxT[(cc, ss)][:, :],
                                            ps[:, :], bq_t[:, cc:cc + 1])

            def emit_g1q(ss):
                for cc in range(4):
                    emit_g1q_cc(ss, cc)

            def emit_attn_pair(ss, pair):
                s0 = ss * 512
                njb = 4 * (ss + 1) if causal else NJB
                qx = qxT[(pair, ss)]
                at = ps_at.tile([128, 1024], F32, tag="at")
                for jb in range(njb):
                    jss, jr = jb // 4, jb % 4
                    off = max(0, jb * 128 - s0) if causal else 0
                    N = 512 - off
                    sp = ps_sp.tile([128, 1024], F32, tag="sp")
                    nc.tensor.matmul(
                        sp[:, 0:N],
                        kxT[jss][0:64, jr * 128:(jr + 1) * 128],
                        qx[0:64, off:512], start=True, stop=True)
                    nc.tensor.matmul(
                        sp[:, 512:512 + N],
                        kxT[jss][64:128, jr * 128:(jr + 1) * 128],
                        qx[64:128, off:512], start=True, stop=True)
                    if mode == "dense":
                        mb = mbp.tile([128, 512], F32, tag="mb")
                        nc.sync.dma_start(
                            out=mb[:, 0:N],
                            in_=mbias[jb * 128:(jb + 1) * 128,
                                      s0 + off:s0 + 512])
                        nc.vector.tensor_tensor(
                            sp[:, 0:N], sp[:, 0:N], mb[:, 0:N], ALU.add)
                        nc.vector.tensor_tensor(
                            sp[:, 512:512 + N], sp[:, 512:512 + N],
                            mb[:, 0:N], ALU.add)
                    ex = expp.tile([128, 1024], BF16, tag="ex")
                    nc.scalar.activation(
                        _ap3(ex[:, 0:1024], 512, 2, N),
                        _ap3(sp[:, 0:1024], 512, 2, N),
                        AF.Exp, scale=0.125)
                    if causal and jss == ss:
                        nc.vector.tensor_tensor(
                            _ap3(ex[:, 0:1024], 512, 2, 128),
                            _ap3(ex[:, 0:1024], 512, 2, 128),
                            _ap3(tri2_t[:, 0:256], 128, 2, 128),
                            ALU.mult)
                    nc.tensor.matmul(
                        at[:, off:512], vx1r[jb][:, 0:128],
                        ex[:, 0:N],
                        start=(jb == 0), stop=(jb == njb - 1))
                    nc.tensor.matmul(
                        at[:, 512 + off:1024], vx1r[jb][:, 128:256],
                        ex[:, 512:512 + N],
                        start=(jb == 0), stop=(jb == njb - 1))
                # normalize: rows 64:128 hold the denominator replicated
                # 64-wide; 1/D = exp(-ln(D)) on ACT (Exp and Ln share one
                # table set; each call ~6x cheaper than DVE InstReciprocal)
                lnD = nmp.tile([64, 1024], F32, tag="nm")
                nc.scalar.activation(lnD[:, :], at[64:128, 0:1024], AF.Ln)
                nm = nmp.tile([64, 1024], F32, tag="nm")
                nc.scalar.activation(nm[:, :], lnD[:, :], AF.Exp,
                                     scale=-1.0)
                aT = attnT[(pair, ss)]
                nc.vector.tensor_tensor(
                    aT[0:64, :], at[0:64, 0:512], nm[0:64, 0:512],
                    ALU.mult)
                nc.vector.tensor_tensor(
                    aT[64:128, :], at[0:64, 512:1024],
                    nm[0:64, 512:1024], ALU.mult)

            def emit_g4(ss, ic):
                s0 = ss * 512
                i0 = ic * 128
                for hf in range(2):
                    ob = obp.tile([128, 1024], F32, tag="ob", name="ob")
                    for e2 in range(2):
                        ec = hf * 2 + e2
                        g4 = ps_pj.tile([128, 512], F32, tag="pj")
                        for cc2 in range(4):
                            nc.tensor.matmul(
                                g4[:, :],
                                attnT[(cc2, ss)][:, i0:i0 + 128],
                                wo_ap(cc2, ec),
                                start=(cc2 == 0), stop=(cc2 == 3))
                        nc.vector.tensor_copy(
                            ob[:, e2 * 512:(e2 + 1) * 512], g4[:, :])
                    nc.sync.dma_start(
                        out=out[s0 + i0:s0 + i0 + 128,
                                hf * 1024:(hf + 1) * 1024],
                        in_=ob[:, :])

            # ---- phase A: k/v projections + v transpose.
            # Loads split across queues so both stream in parallel:
            # sync(HWDGE): wkv, kT sh0, wq, kT sh1, wo;
            # gpsimd(SWDGE): vT sh0, qT sh0, vT sh1.
            # v-chains run before k-chains (vT lands first). ----
            kvt = {}
            nc.sync.dma_start(out=wq_sb[:, :], in_=wq[:, :])
            for dc in range(NDC):
                t = stg.tile([128, 1024], BF16, tag="kv", name="kvstg")
                nc.gpsimd.dma_start(
                    out=t[:, :], in_=vT[dc * 128:(dc + 1) * 128, 0:1024])
                kvt[("v", dc, 0)] = t
            for dc in range(NDC):
                t = stg.tile([128, 1024], BF16, tag="kv", name="kvstg")
                nc.sync.dma_start(
                    out=t[:, :], in_=kT[dc * 128:(dc + 1) * 128, 0:1024])
                kvt[("k", dc, 0)] = t
            # qT sh0 split across both queues so the first qxT chain can
            # start as early as possible
            for dc in range(NDC):
                t = qtp.tile([128, 1024], BF16, tag=f"qT{dc}",
                             name=f"qT{dc}_0")
                eng = nc.sync if dc < 8 else nc.gpsimd
                eng.dma_start(out=t[:, :],
                              in_=qT[dc * 128:(dc + 1) * 128, 0:1024])
                qT_t[(dc, 0)] = t

            def load_kv_sh1():
                # sh=1 tiles reuse the sh=0 slots, so this is emitted
                # only after the ss0/ss1 chains that release them
                for dc in range(NDC):
                    t = stg.tile([128, 1024], BF16, tag="kv", name="kvstg")
                    nc.sync.dma_start(
                        out=t[:, :],
                        in_=kT[dc * 128:(dc + 1) * 128, 1024:2048])
                    kvt[("k", dc, 1)] = t
                for dc in range(NDC):
                    t = stg.tile([128, 1024], BF16, tag="kv", name="kvstg")
                    nc.gpsimd.dma_start(
                        out=t[:, :],
                        in_=vT[dc * 128:(dc + 1) * 128, 1024:2048])
                    kvt[("v", dc, 1)] = t
                nc.sync.dma_start(out=wo_sb[:, :], in_=wo[:, :])

            def emit_kv_chain(ss, which):
                sh, lo = ss // 2, (ss % 2) * 512
                if which == "v":
                    ps = ps_pj.tile([128, 512], F32, tag="pj")
                    for dc in range(NDC):
                        nc.tensor.matmul(ps[:, :], wv_ap(dc),
                                         kvt[("v", dc, sh)][:, lo:lo + 512],
                                         start=(dc == 0),
                                         stop=(dc == NDC - 1))
                    vsb = vxsb.tile([128, 512], BF16, tag="vsb")
                    nc.vector.tensor_scalar_add(vsb[:, :], ps[:, :],
                                                bv_t[:, 0:1])
                    vtp = ps_pj.tile([128, 512], BF16, tag="pj")
                    for jr in range(4):
                        nc.tensor.transpose(
                            vtp[:, jr * 128:(jr + 1) * 128],
                            vsb[:, jr * 128:(jr + 1) * 128], id_t[:, :])
                    for jr in range(4):
                        jb = ss * 4 + jr
                        nc.vector.tensor_copy(
                            vx1r[jb][:, 0:64],
                            vtp[:, jr * 128:jr * 128 + 64])
                        nc.vector.tensor_copy(
                            vx1r[jb][:, 128:192],
                            vtp[:, jr * 128 + 64:jr * 128 + 128])
                else:
                    ps = ps_pj.tile([128, 512], F32, tag="pj")
                    for dc in range(NDC):
                        nc.tensor.matmul(ps[:, :], wk_ap(dc),
                                         kvt[("k", dc, sh)][:, lo:lo + 512],
                                         start=(dc == 0),
                                         stop=(dc == NDC - 1))
                    nc.vector.tensor_scalar_add(kxT[ss][:, :], ps[:, :],
                                                bk_t[:, 0:1])

            # ---- driver: the ACT exp/norm stream is the serialized
            # critical path (~230us), so attention pairs start as early
            # as possible (just-in-time qxT chains) and ALL other PE
            # work -- k/v chains, GEMM4, second-half loads -- is emitted
            # between pairs as fill. Tile-slot grants follow emission
            # order, so fills sit next to attention in priority and get
            # pulled into the PE idle slots where AV waits on Exp. ----
            emit_kv_chain(0, "k")
            emit_kv_chain(0, "v")
            for ss in range(NSS):
                for pair in range(4):
                    emit_g1q_cc(ss, pair)
                    emit_attn_pair(ss, pair)
                    if ss == 0:
                        if pair == 0:
                            emit_kv_chain(1, "k")
                        elif pair == 1:
                            emit_kv_chain(1, "v")
                        elif pair == 2:
                            load_kv_sh1()
                            load_qT(1, nc.gpsimd)
                    elif ss == 1:
                        emit_g4(0, pair)
                        emit_kv_chain(2 + pair // 2, "kv"[pair % 2])
                    else:
                        emit_g4(ss - 1, pair)
            for ic in range(4):
                emit_g4(NSS - 1, ic)
    nc.finalize()
    return nc


_CACHE = {}


def _get_nc(mode):
    if mode not in _CACHE:
        _CACHE[mode] = build_nc(mode)
    return _CACHE[mode]


def kernel(q, k, v, mask, Wq, bq, Wk, bk, Wv, bv, Wo, bo):
    bf = ml_dtypes.bfloat16
    q = np.asarray(q, np.float32)
    k = np.asarray(k, np.float32)
    v = np.asarray(v, np.float32)
    mask = np.asarray(mask)
    Wq = np.asarray(Wq, np.float32)
    Wk = np.asarray(Wk, np.float32)
    Wv = np.asarray(Wv, np.float32)
    Wo = np.asarray(Wo, np.float32)
    bq = np.asarray(bq, np.float32)
    bk = np.asarray(bk, np.float32)
    bv = np.asarray(bv, np.float32)
    bo = np.asarray(bo, np.float32)

    m = mask.astype(np.float64)
    if np.array_equal(m, np.tril(np.ones((S, S)))):
        mode = "causal"
    elif np.all(m == 1):
        mode = "none"
    else:
        mode = "dense"

    nc = _get_nc(mode)
    tri = np.triu(np.ones((128, 128), np.float32))
    tri2_np = np.concatenate([tri, tri], axis=1).astype(bf)
    id_np = np.eye(128).astype(bf)

    # On-chip layout places local q head h in tile h%4 at partition
    # (h//4)*64 so q/k partition bases match in the scores matmul. Permute
    # Wq columns / Wo rows / bq accordingly: tile cc holds heads (cc, cc+4).
    head_perm = [h for cc in range(4) for h in (cc, cc + 4)]
    col_perm = np.concatenate(
        [np.arange(h * HD, (h + 1) * HD) for h in head_perm])

    # per-batch transposed bf16 inputs (shared across the 4 kv shards)
    qT_b = [np.ascontiguousarray(q[b].astype(bf).T) for b in range(B)]
    kT_b = [np.ascontiguousarray(k[b].astype(bf).T) for b in range(B)]
    vT_b = [np.ascontiguousarray(v[b].astype(bf).T) for b in range(B)]

    in_maps = []
    for core in range(NCORES):
        b, kb = core // KVSH, core % KVSH
        wq_sh = Wq[:, kb * CQ:(kb + 1) * CQ][:, col_perm]
        wo_sh = Wo[kb * CQ:(kb + 1) * CQ, :][col_perm, :]
        bq_sh = bq[kb * CQ:(kb + 1) * CQ][col_perm]
        # pre-gather weights into the on-chip layout (dc-major columns)
        wq_g = wq_sh.astype(bf).reshape(NDC, 128, CQ).transpose(1, 0, 2)
        wk_r = Wk[:, kb * CK:(kb + 1) * CK].astype(bf).reshape(NDC, 128, CK)
        wv_r = Wv[:, kb * CK:(kb + 1) * CK].astype(bf).reshape(NDC, 128, CK)
        wkv_g = np.concatenate([wk_r, wv_r], axis=2).transpose(1, 0, 2)
        wo_g = wo_sh.astype(bf).reshape(4, 128, DIM).transpose(1, 0, 2)
        im = {
            "qT": qT_b[b],
            "kT": kT_b[b],
            "vT": vT_b[b],
            "wq": np.ascontiguousarray(wq_g.reshape(128, NDC * CQ)),
            "wkv": np.ascontiguousarray(wkv_g.reshape(128, NDC * 2 * CK)),
            "wo": np.ascontiguousarray(wo_g.reshape(128, 4 * DIM)),
            "bq": np.ascontiguousarray(bq_sh),
            "bk": np.ascontiguousarray(bk[kb * CK:(kb + 1) * CK]),
            "bv": np.ascontiguousarray(bv[kb * CK:(kb + 1) * CK]),
            "tri2": tri2_np,
            "ident": id_np,
        }
        if mode == "dense":
            with np.errstate(divide="ignore"):
                bias = -(1.0 / mask.astype(np.float32) + 1.0)
            im["mbias"] = np.ascontiguousarray(bias.T * 8.0)
        in_maps.append(im)

    res = run_bass_kernel_spmd(nc, in_maps, core_ids=list(range(NCORES)))
    outs = [r["out"] for r in res.results]
    full = np.empty((B, S, DIM), np.float32)
    for b in range(B):
        acc = outs[b * KVSH].astype(np.float32)
        for kb in range(1, KVSH):
            acc = acc + outs[b * KVSH + kb]
        full[b] = acc + bo[None, :]
    return full


# revision 26
# speedup vs baseline: 1.1469x; 1.0069x over previous
"""Grouped-query attention (GQA) Trainium2 Bass kernel, v2.

Problem: B=2, S=2048, DIM=2048, HQ=32, HKV=8, HEAD_DIM=64, causal mask.
Sharding: 8 cores = 2 (batch) x 4 (kv-head groups). Core c handles batch
c//4 and kv-block c%4 (2 kv heads, 8 q heads). Wq/Wk/Wv sharded
column-wise, Wo row-wise; each core writes a partial [S, DIM] output;
host sums the 4 partials per batch and adds bo.

v2 dataflow (all matmuls bf16 with fp32 PSUM accum):
  - q/k/v are transposed AND cast to bf16 on the HOST -> qT/kT/vT
    [DIM, S] in HBM. No on-chip input transposes or casts; DMA traffic
    halves vs f32 naturals.
  - Weights pre-cast to bf16 on host (columns of Wq / rows of Wo
    permuted so local q-head h sits in tile h%4 at partition (h//4)*64,
    matching its kv head's partition base in kxT).
  - Projections: kxT/vxT first (phase A), then per 512-row i-block:
    qxT, attention, output projection. Biases added on DVE
    (tensor_scalar) during PSUM->SBUF eviction.
  - Scores: the two kv heads of a q-head pair run as row-tiled
    concurrent matmuls (K=64 each, PE row halves 0-63 / 64-127) into
    one 2-bank PSUM tile; ONE Exp activation covers both (3-D AP skips
    the causally-masked tail). Triangular mask applied multiplicatively
    post-exp on diagonal j-blocks only; j-blocks above the diagonal are
    skipped entirely.
  - AV: stationary is [v_head (64 cols) | ones (64 cols)], so PSUM rows
    64:127 accumulate the softmax denominator replicated 64-wide.
    Normalization = full-width DVE reciprocal + 2 multiplies (no DMA
    broadcast, no single-partition ops).
  - Output projection: fp32 partial written straight from a [128, 2048]
    SBUF staging tile, 1 MiB per DMA.
"""

import numpy as np
import ml_dtypes

import concourse.bass as bass
import concourse.mybir as mybir
from concourse import bacc
from concourse.tile import TileContext
from concourse.bass_utils import run_bass_kernel_spmd

# This kernel uses Exp (softmax) and Ln (denominator reciprocal via
# exp(-ln(D))) on the ACT engine. The table-load placement pass maps each
# function to the first table set containing it, which picks
# `exp_and_others` for Exp and `natural_log` for Ln and thrashes
# ACT_TABLE_LOADs (~1.3us + drain each) on every normalization. Both live
# in `natural_log_exp_and_others`; narrow the claimed contents of the
# other sets (names and dict order - hence set ids - are unchanged) so
# the pass settles on the shared set once.
_orig_get_act_tables = bacc.get_activation_tables


def _pinned_act_tables(arch):
    tabs = _orig_get_act_tables(arch)
    exp = mybir.ActivationFunctionType.Exp
    ln = mybir.ActivationFunctionType.Ln
    shared = "natural_log_exp_and_others"
    if shared in tabs and exp in tabs[shared] and ln in tabs[shared]:
        for name, funcs in tabs.items():
            if name != shared:
                tabs[name] = funcs - {exp, ln}
    return tabs


bacc.get_activation_tables = _pinned_act_tables

F32 = mybir.dt.float32
BF16 = mybir.dt.bfloat16
AF = mybir.ActivationFunctionType
ALU = mybir.AluOpType

B, S, DIM = 2, 2048, 2048
HQ, HKV, HD = 32, 8, 64
GROUP = HQ // HKV              # 4
NCORES = 8
KVSH = 4                       # kv-blocks (shards) per batch
CQ = (HQ // KVSH) * HD         # 512 q-proj cols per core (8 heads)
CK = (HKV // KVSH) * HD        # 128 kv-proj cols per core (2 heads)
NDC = DIM // 128               # 16 contraction chunks
NSS = S // 512                 # 4 sequence chunks of 512
NJB = S // 128                 # 16 j-blocks of 128


def _ap3(sl, mid_stride, mid_n, last_n):
    """3-D AP over a 2-D tile slice: [partitions, mid_n x mid_stride,
    last_n] (element strides)."""
    return bass.AP(tensor=sl.tensor, offset=sl.offset,
                   ap=[list(sl.ap[0]), [mid_stride, mid_n], [1, last_n]])


def build_nc(mode="causal"):
    nc = bacc.Bacc("TRN2", target_bir_lowering=False)

    qT = nc.dram_tensor("qT", [DIM, S], BF16, kind="ExternalInput")
    kT = nc.dram_tensor("kT", [DIM, S], BF16, kind="ExternalInput")
    vT = nc.dram_tensor("vT", [DIM, S], BF16, kind="ExternalInput")
    # host pre-gathers weights into SBUF layout: one DMA per tensor
    wq = nc.dram_tensor("wq", [128, NDC * CQ], BF16, kind="ExternalInput")
    wkv = nc.dram_tensor("wkv", [128, NDC * 2 * CK], BF16,
                         kind="ExternalInput")
    wo = nc.dram_tensor("wo", [128, 4 * DIM], BF16, kind="ExternalInput")
    # packed constants: biases [128, 6] f32 (bq cc0-3 | bk | bv) and
    # tri2+ident [128, 384] bf16 - one DMA each off the critical path
    bqkv = nc.dram_tensor("bqkv", [128, 6], F32, kind="ExternalInput")
    trid = nc.dram_tensor("trid", [128, 384], BF16, kind="ExternalInput")
    mbias = None
    if mode == "dense":
        mbias = nc.dram_tensor("mbias", [S, S], F32, kind="ExternalInput")
    out = nc.dram_tensor("out", [S, DIM], F32, kind="ExternalOutput")

    causal = mode == "causal"

    with TileContext(nc) as tc:
        with (
            tc.tile_pool(name="consts", bufs=1) as consts,
            tc.tile_pool(name="w", bufs=1) as wpool,
            tc.tile_pool(name="qt", bufs=1) as qtp,
            tc.tile_pool(name="stg", bufs=32) as stg,
            tc.tile_pool(name="acts", bufs=1) as acts,
            tc.tile_pool(name="vsb", bufs=2) as vxsb,
            tc.tile_pool(name="exp", bufs=4) as expp,
            tc.tile_pool(name="nm", bufs=2) as nmp,
            tc.tile_pool(name="ob", bufs=2) as obp,
            tc.tile_pool(name="mb", bufs=2) as mbp,
            tc.tile_pool(name="ps_sp", bufs=2, space="PSUM") as ps_sp,
            tc.tile_pool(name="ps_at", bufs=1, space="PSUM") as ps_at,
            tc.tile_pool(name="ps_pj", bufs=2, space="PSUM") as ps_pj,
        ):
            # ---- constants: two packed DMAs ----
            bqkv_t = consts.tile([128, 6], F32, tag="bqkv")
            nc.sync.dma_start(out=bqkv_t[:, :], in_=bqkv[:, :])
            trid_t = consts.tile([128, 384], BF16, tag="trid")
            nc.sync.dma_start(out=trid_t[:, :], in_=trid[:, :])

            # ---- weights: one DMA each (host pre-gathered layout) ----
            wkv_sb = wpool.tile([128, NDC * 2 * CK], BF16, tag="wkv")
            nc.sync.dma_start(out=wkv_sb[:, :], in_=wkv[:, :])

            def wk_ap(dc):
                return wkv_sb[:, dc * 256:dc * 256 + 128]

            def wv_ap(dc):
                return wkv_sb[:, dc * 256 + 128:(dc + 1) * 256]

            wq_sb = wpool.tile([128, NDC * CQ], BF16, tag="wq")

            def wq_ap(dc, cc):
                return wq_sb[:, dc * CQ + cc * 128:dc * CQ + (cc + 1) * 128]

            wo_sb = wpool.tile([128, 4 * DIM], BF16, tag="wo")

            def wo_ap(cc, ec):
                return wo_sb[:, cc * DIM + ec * 512:cc * DIM + (ec + 1) * 512]

            # ---- qT loads: [128, 1024] per (dc, half). sh=0 rides the
            # SWDGE queue behind vT sh0; sh=1 is emitted at its phase-B
            # use point on HWDGE (slot grants follow emission order, so
            # an up-front emit would deadlock). ----
            qT_t = {}

            def load_qT(sh, eng):
                for dc in range(NDC):
                    t = qtp.tile([128, 1024], BF16, tag=f"qT{dc}",
                                 name=f"qT{dc}_{sh}")
                    eng.dma_start(
                        out=t[:, :],
                        in_=qT[dc * 128:(dc + 1) * 128,
                               sh * 1024:(sh + 1) * 1024])
                    qT_t[(dc, sh)] = t

            # ---- persistent activations ----
            qxT = {}   # (cc, ss) -> [128, 512]; rows 0:64 head cc (kv0),
            #            rows 64:128 head cc+4 (kv1)
            kxT = {}   # ss -> [128, 512]
            attnT = {}  # (pair, ss) -> [128, 512]
            for ss in range(NSS):
                kxT[ss] = acts.tile([128, 512], BF16, tag=f"kx{ss}", name=f"kx{ss}")
                for cc in range(4):
                    qxT[(cc, ss)] = acts.tile([128, 512], BF16,
                                              tag=f"qx{cc}_{ss}", name=f"qx{cc}_{ss}")
                    attnT[(cc, ss)] = acts.tile([128, 512], BF16,
                                                tag=f"at{cc}_{ss}", name=f"at{cc}_{ss}")
            # [v_head | 64 ones cols] per kv head, per j-block
            vx1r = []
            for jb in range(NJB):
                t = acts.tile([128, 256], BF16, tag=f"vp{jb}", name=f"vp{jb}")
                nc.vector.memset(t[:, 64:128], 1.0)
                nc.vector.memset(t[:, 192:256], 1.0)
                vx1r.append(t)

            # ---- phase B helpers ----
            def emit_g1q_cc(ss, cc):
                sh, lo = ss // 2, (ss % 2) * 512
                ps = ps_pj.tile([128, 512], F32, tag="pj")
                for dc in range(NDC):
                    nc.tensor.matmul(
                        ps[:, :], wq_ap(dc, cc),
                        qT_t[(dc, sh)][:, lo:lo + 512],
                        start=(dc == 0), stop=(dc == NDC - 1))
                nc.vector.tensor_scalar_add(qxT[(cc, ss)][:, :],
                                            ps[:, :], bqkv_t[:, cc:cc + 1])

            def emit_g1q(ss):
                for cc in range(4):
                    emit_g1q_cc(ss, cc)

            def emit_attn_pair(ss, pair):
                s0 = ss * 512
                njb = 4 * (ss + 1) if causal else NJB
                qx = qxT[(pair, ss)]
                at = ps_at.tile([128, 1024], F32, tag="at")
                for jb in range(njb):
                    jss, jr = jb // 4, jb % 4
                    off = max(0, jb * 128 - s0) if causal else 0
                    N = 512 - off
                    sp = ps_sp.tile([128, 1024], F32, tag="sp")
                    nc.tensor.matmul(
                        sp[:, 0:N],
                        kxT[jss][0:64, jr * 128:(jr + 1) * 128],
                        qx[0:64, off:512], start=True, stop=True)
                    nc.tensor.matmul(
                        sp[:, 512:512 + N],
                        kxT[jss][64:128, jr * 128:(jr + 1) * 128],
                        qx[64:128, off:512], start=True, stop=True)
                    if mode == "dense":
                        mb = mbp.tile([128, 512], F32, tag="mb")
                        nc.sync.dma_start(
                            out=mb[:, 0:N],
                            in_=mbias[jb * 128:(jb + 1) * 128,
                                      s0 + off:s0 + 512])
                        nc.vector.tensor_tensor(
                            sp[:, 0:N], sp[:, 0:N], mb[:, 0:N], ALU.add)
                        nc.vector.tensor_tensor(
                            sp[:, 512:512 + N], sp[:, 512:512 + N],
                            mb[:, 0:N], ALU.add)
                    ex = expp.tile([128, 1024], BF16, tag="ex")
                    nc.scalar.activation(
                        _ap3(ex[:, 0:1024], 512, 2, N),
                        _ap3(sp[:, 0:1024], 512, 2, N),
                        AF.Exp, scale=0.125)
                    if causal and jss == ss:
                        nc.vector.tensor_tensor(
                            _ap3(ex[:, 0:1024], 512, 2, 128),
                            _ap3(ex[:, 0:1024], 512, 2, 128),
                            _ap3(trid_t[:, 0:256], 128, 2, 128),
                            ALU.mult)
                    nc.tensor.matmul(
                        at[:, off:512], vx1r[jb][:, 0:128],
                        ex[:, 0:N],
                        start=(jb == 0), stop=(jb == njb - 1))
                    nc.tensor.matmul(
                        at[:, 512 + off:1024], vx1r[jb][:, 128:256],
                        ex[:, 512:512 + N],
                        start=(jb == 0), stop=(jb == njb - 1))
                # normalize: rows 64:128 hold the denominator replicated
                # 64-wide; 1/D = exp(-ln(D)) on ACT (Exp and Ln share one
                # table set; each call ~6x cheaper than DVE InstReciprocal)
                lnD = nmp.tile([64, 1024], F32, tag="nm")
                nc.scalar.activation(lnD[:, :], at[64:128, 0:1024], AF.Ln)
                nm = nmp.tile([64, 1024], F32, tag="nm")
                nc.scalar.activation(nm[:, :], lnD[:, :], AF.Exp,
                                     scale=-1.0)
                aT = attnT[(pair, ss)]
                nc.vector.tensor_tensor(
                    aT[0:64, :], at[0:64, 0:512], nm[0:64, 0:512],
                    ALU.mult)
                nc.vector.tensor_tensor(
                    aT[64:128, :], at[0:64, 512:1024],
                    nm[0:64, 512:1024], ALU.mult)

            def emit_g4(ss, ic):
                s0 = ss * 512
                i0 = ic * 128
                for hf in range(2):
                    ob = obp.tile([128, 1024], F32, tag="ob", name="ob")
                    for e2 in range(2):
                        ec = hf * 2 + e2
                        g4 = ps_pj.tile([128, 512], F32, tag="pj")
                        for cc2 in range(4):
                            nc.tensor.matmul(
                                g4[:, :],
                                attnT[(cc2, ss)][:, i0:i0 + 128],
                                wo_ap(cc2, ec),
                                start=(cc2 == 0), stop=(cc2 == 3))
                        nc.vector.tensor_copy(
                            ob[:, e2 * 512:(e2 + 1) * 512], g4[:, :])
                    nc.sync.dma_start(
                        out=out[s0 + i0:s0 + i0 + 128,
                                hf * 1024:(hf + 1) * 1024],
                        in_=ob[:, :])

            # ---- phase A: k/v projections + v transpose.
            # Loads split across queues so both stream in parallel:
            # sync(HWDGE): wkv, kT sh0, wq, kT sh1, wo;
            # gpsimd(SWDGE): vT sh0, qT sh0, vT sh1.
            # v-chains run before k-chains (vT lands first). ----
            kvt = {}
            nc.sync.dma_start(out=wq_sb[:, :], in_=wq[:, :])
            for dc in range(NDC):
                t = stg.tile([128, 1024], BF16, tag="kv", name="kvstg")
                nc.gpsimd.dma_start(
                    out=t[:, :], in_=vT[dc * 128:(dc + 1) * 128, 0:1024])
                kvt[("v", dc, 0)] = t
            for dc in range(NDC):
                t = stg.tile([128, 1024], BF16, tag="kv", name="kvstg")
                nc.sync.dma_start(
                    out=t[:, :], in_=kT[dc * 128:(dc + 1) * 128, 0:1024])
                kvt[("k", dc, 0)] = t
            # qT sh0 split across both queues so the first qxT chain can
            # start as early as possible
            for dc in range(NDC):
                t = qtp.tile([128, 1024], BF16, tag=f"qT{dc}",
                             name=f"qT{dc}_0")
                eng = nc.sync if dc < 8 else nc.gpsimd
                eng.dma_start(out=t[:, :],
                              in_=qT[dc * 128:(dc + 1) * 128, 0:1024])
                qT_t[(dc, 0)] = t

            def load_kv_sh1():
                # sh=1 tiles reuse the sh=0 slots, so this is emitted
                # only after the ss0/ss1 chains that release them
                for dc in range(NDC):
                    t = stg.tile([128, 1024], BF16, tag="kv", name="kvstg")
                    nc.sync.dma_start(
                        out=t[:, :],
                        in_=kT[dc * 128:(dc + 1) * 128, 1024:2048])
                    kvt[("k", dc, 1)] = t
                for dc in range(NDC):
                    t = stg.tile([128, 1024], BF16, tag="kv", name="kvstg")
                    nc.gpsimd.dma_start(
                        out=t[:, :],
                        in_=vT[dc * 128:(dc + 1) * 128, 1024:2048])
                    kvt[("v", dc, 1)] = t
                nc.sync.dma_start(out=wo_sb[:, :], in_=wo[:, :])

            def emit_kv_chain(ss, which):
                sh, lo = ss // 2, (ss % 2) * 512
                if which == "v":
                    ps = ps_pj.tile([128, 512], F32, tag="pj")
                    for dc in range(NDC):
                        nc.tensor.matmul(ps[:, :], wv_ap(dc),
                                         kvt[("v", dc, sh)][:, lo:lo + 512],
                                         start=(dc == 0),
                                         stop=(dc == NDC - 1))
                    vsb = vxsb.tile([128, 512], BF16, tag="vsb")
                    nc.vector.tensor_scalar_add(vsb[:, :], ps[:, :],
                                                bqkv_t[:, 5:6])
                    vtp = ps_pj.tile([128, 512], BF16, tag="pj")
                    for jr in range(4):
                        nc.tensor.transpose(
                            vtp[:, jr * 128:(jr + 1) * 128],
                            vsb[:, jr * 128:(jr + 1) * 128], trid_t[:, 256:384])
                    for jr in range(4):
                        jb = ss * 4 + jr
                        nc.vector.tensor_copy(
                            vx1r[jb][:, 0:64],
                            vtp[:, jr * 128:jr * 128 + 64])
                        nc.vector.tensor_copy(
                            vx1r[jb][:, 128:192],
                            vtp[:, jr * 128 + 64:jr * 128 + 128])
                else:
                    ps = ps_pj.tile([128, 512], F32, tag="pj")
                    for dc in range(NDC):
                        nc.tensor.matmul(ps[:, :], wk_ap(dc),
                                         kvt[("k", dc, sh)][:, lo:lo + 512],
                                         start=(dc == 0),
                                         stop=(dc == NDC - 1))
                    nc.vector.tensor_scalar_add(kxT[ss][:, :], ps[:, :],
                                                bqkv_t[:, 4:5])

            # ---- driver: the ACT exp/norm stream is the serialized
            # critical path (~230us), so attention pairs start as early
            # as possible (just-in-time qxT chains) and ALL other PE
            # work -- k/v chains, GEMM4, second-half loads -- is emitted
            # between pairs as fill. Tile-slot grants follow emission
            # order, so fills sit next to attention in priority and get
            # pulled into the PE idle slots where AV waits on Exp. ----
            emit_kv_chain(0, "k")
            emit_kv_chain(0, "v")
            for ss in range(NSS):
                for pair in range(4):
                    emit_g1q_cc(ss, pair)
                    emit_attn_pair(ss, pair)
                    if ss == 0:
                        if pair == 0:
                            emit_kv_chain(1, "k")
                        elif pair == 1:
                            emit_kv_chain(1, "v")
                        elif pair == 2:
                            load_kv_sh1()
                            load_qT(1, nc.gpsimd)
                    elif ss == 1:
                        emit_g4(0, pair)
                        emit_kv_chain(2 + pair // 2, "kv"[pair % 2])
                    else:
                        emit_g4(ss - 1, pair)
            for ic in range(4):
                emit_g4(NSS - 1, ic)
    nc.finalize()
    return nc


_CACHE = {}


def _get_nc(mode):
    if mode not in _CACHE:
        _CACHE[mode] = build_nc(mode)
    return _CACHE[mode]


def kernel(q, k, v, mask, Wq, bq, Wk, bk, Wv, bv, Wo, bo):
    bf = ml_dtypes.bfloat16
    q = np.asarray(q, np.float32)
    k = np.asarray(k, np.float32)
    v = np.asarray(v, np.float32)
    mask = np.asarray(mask)
    Wq = np.asarray(Wq, np.float32)
    Wk = np.asarray(Wk, np.float32)
    Wv = np.asarray(Wv, np.float32)
    Wo = np.asarray(Wo, np.float32)
    bq = np.asarray(bq, np.float32)
    bk = np.asarray(bk, np.float32)
    bv = np.asarray(bv, np.float32)
    bo = np.asarray(bo, np.float32)

    m = mask.astype(np.float64)
    if np.array_equal(m, np.tril(np.ones((S, S)))):
        mode = "causal"
    elif np.all(m == 1):
        mode = "none"
    else:
        mode = "dense"

    nc = _get_nc(mode)
    tri = np.triu(np.ones((128, 128), np.float32))
    trid_np = np.concatenate([tri, tri, np.eye(128, dtype=np.float32)],
                             axis=1).astype(bf)

    # On-chip layout places local q head h in tile h%4 at partition
    # (h//4)*64 so q/k partition bases match in the scores matmul. Permute
    # Wq columns / Wo rows / bq accordingly: tile cc holds heads (cc, cc+4).
    head_perm = [h for cc in range(4) for h in (cc, cc + 4)]
    col_perm = np.concatenate(
        [np.arange(h * HD, (h + 1) * HD) for h in head_perm])

    # per-batch transposed bf16 inputs (shared across the 4 kv shards)
    qT_b = [np.ascontiguousarray(q[b].astype(bf).T) for b in range(B)]
    kT_b = [np.ascontiguousarray(k[b].astype(bf).T) for b in range(B)]
    vT_b = [np.ascontiguousarray(v[b].astype(bf).T) for b in range(B)]

    in_maps = []
    for core in range(NCORES):
        b, kb = core // KVSH, core % KVSH
        wq_sh = Wq[:, kb * CQ:(kb + 1) * CQ][:, col_perm]
        wo_sh = Wo[kb * CQ:(kb + 1) * CQ, :][col_perm, :]
        bq_sh = bq[kb * CQ:(kb + 1) * CQ][col_perm]
        # pre-gather weights into the on-chip layout (dc-major columns)
        wq_g = wq_sh.astype(bf).reshape(NDC, 128, CQ).transpose(1, 0, 2)
        wk_r = Wk[:, kb * CK:(kb + 1) * CK].astype(bf).reshape(NDC, 128, CK)
        wv_r = Wv[:, kb * CK:(kb + 1) * CK].astype(bf).reshape(NDC, 128, CK)
        wkv_g = np.concatenate([wk_r, wv_r], axis=2).transpose(1, 0, 2)
        wo_g = wo_sh.astype(bf).reshape(4, 128, DIM).transpose(1, 0, 2)
        im = {
            "qT": qT_b[b],
            "kT": kT_b[b],
            "vT": vT_b[b],
            "wq": np.ascontiguousarray(wq_g.reshape(128, NDC * CQ)),
            "wkv": np.ascontiguousarray(wkv_g.reshape(128, NDC * 2 * CK)),
            "wo": np.ascontiguousarray(wo_g.reshape(128, 4 * DIM)),
            "bqkv": np.ascontiguousarray(np.concatenate(
                [bq_sh.reshape(4, 128).T,
                 bk[kb * CK:(kb + 1) * CK].reshape(128, 1),
                 bv[kb * CK:(kb + 1) * CK].reshape(128, 1)], axis=1)),
            "trid": trid_np,
        }
        if mode == "dense":
            with np.errstate(divide="ignore"):
                bias = -(1.0 / mask.astype(np.float32) + 1.0)
            im["mbias"] = np.ascontiguousarray(bias.T * 8.0)
        in_maps.append(im)

    res = run_bass_kernel_spmd(nc, in_maps, core_ids=list(range(NCORES)))
    outs = [r["out"] for r in res.results]
    full = np.empty((B, S, DIM), np.float32)
    for b in range(B):
        acc = outs[b * KVSH].astype(np.float32)
        for kb in range(1, KVSH):
            acc = acc + outs[b * KVSH + kb]
        full[b] = acc + bo[None, :]
    return full
